# revision 1
# baseline (speedup 1.0000x reference)
"""Trainium2 Bass kernel for nn_GedLayer (graph edit distance forward).

The reference builds a 9216x9216 cost matrix C whose entries are a 4x4
lookup T[A1[i,j], A2[k,l]] over edge-label pairs, then computes
    ged = 0.5 * v @ (Dmat @ v) + c @ v
with v = vec(S) from a 10-iteration Sinkhorn on the 96x96 node-cost grid.

Because edge labels take only 4 values, the quadratic form factorizes into
96x96 matmuls (no 9216^2 matrix is ever formed):
    Zt[k,(q,i)] = sum_j S'[j,k] P_q[j,i]          one wide 96x96x384 matmul
    F[i,l]      = sum_qk Zt[k,(q,i)] C[k] B2_q[k,l]   4 PSUM-accum matmuls
    v' C0 v     = sum_il F[i,l] S'[i,l] C[l]
where P_q/B2_q are host-built indicator lookups of the int edge matrices,
S' = diag(R) S0, and (R, C) come from Sinkhorn run in vector form
(R = 1/(S0m' C), C = 1/(S0Tm' R); the "last scale pinned to 1" rule is
implemented by baking an e_95 column into the matvec operands so a
full-tile reciprocal preserves the pin). All arithmetic on device is fp32.

Sharding: one graph pair, strictly serial Sinkhorn recursion -> the
problem is latency-bound at 96x96 scale, so the computation is replicated
on all 8 cores (SPMD) and core 0's output is returned.
"""

import numpy as np
from contextlib import ExitStack

import concourse.bass as bass
import concourse.tile as tile
from concourse import mybir
from concourse.bass_utils import run_bass_kernel_spmd

NB_LABELS = 10
NB_EDGE_LABELS = 3
SINKHORN_ITERS = 10
L = NB_EDGE_LABELS + 1
N1 = 96
F32 = mybir.dt.float32
N_CORES = 8

_NC_CACHE = {}


def _legalize_waits(nc):
    """Split multi-sem waits into standalone EventSemaphore instructions
    (this walrus codegen fits one sync wait per lowered instruction)."""
    n = 0
    for f in nc.m.functions:
        for bb in f.blocks:
            out = []
            for ins in bb.instructions:
                si = ins.sync_info
                waits = list(si.on_wait) if (si and si.on_wait) else []
                if len(waits) > 1:
                    for w in waits[:-1]:
                        n += 1
                        out.append(mybir.InstEventSemaphore(
                            name=f"LW-{n}",
                            engine=ins.engine,
                            ins=[],
                            outs=[],
                            sync_info=mybir.SyncInfo(on_wait=[w], on_update=[]),
                        ))
                    si.on_wait = [waits[-1]]
                out.append(ins)
            bb.instructions = out
    return n


def _build_nc(legalize=True):
    nc = bass.Bass()
    # grids = [cgrid | cgmod | cgTmod | ddiag] along the free dim
    g_d = nc.dram_tensor("grids", [N1, 4, N1], F32, kind="ExternalInput")
    # tabs = [pmat (j,q,i) | b2 (k,q,l)] along the free dim
    t_d = nc.dram_tensor("tabs", [N1, 2, L, N1], F32, kind="ExternalInput")
    out_d = nc.dram_tensor("out", [1, 1], F32, kind="ExternalOutput")

    ExpF = mybir.ActivationFunctionType.Exp
    mult = mybir.AluOpType.mult
    add = mybir.AluOpType.add

    with tile.TileContext(nc) as tc, ExitStack() as ctx:
        sb = ctx.enter_context(tc.tile_pool(name="sb", bufs=1))

        grids = sb.tile([N1, 4, N1], F32)
        nc.sync.dma_start(out=grids[:], in_=g_d[:])
        tabs = sb.tile([N1, 2, L, N1], F32)
        nc.scalar.dma_start(out=tabs[:], in_=t_d[:])
        cg = grids[:, 0, :]
        cgm = grids[:, 1, :]
        cgTm = grids[:, 2, :]
        dd = grids[:, 3, :]
        pmall = tabs[:, 0, :, :].rearrange("p q i -> p (q i)")
        b2all = tabs[:, 1, :, :]

        ones_col = sb.tile([N1, 1], F32)
        nc.vector.memset(ones_col[:], 1.0)

        s0 = sb.tile([N1, N1], F32)
        nc.scalar.activation(out=s0[:], in_=cg, func=ExpF, scale=-0.5)
        s0m = sb.tile([N1, N1], F32)
        nc.scalar.activation(out=s0m[:], in_=cgm, func=ExpF, scale=-0.5)
        s0Tm = sb.tile([N1, N1], F32)
        nc.scalar.activation(out=s0Tm[:], in_=cgTm, func=ExpF, scale=-0.5)

        # Sinkhorn (see kernel.py): fresh R/C tiles per iteration, pin via
        # the e_95 column baked into cgmod/cgTmod.
        rc = ctx.enter_context(tc.tile_pool(name="rc", bufs=3))
        Cv = rc.tile([N1, 1], F32, tag="c")
        nc.vector.memset(Cv[:], 1.0)
        Rv = None

        with tc.tile_pool(name="mv", bufs=2, space="PSUM") as mv:
            for _ in range(SINKHORN_ITERS):
                u = mv.tile([N1, 1], F32, tag="mv")
                nc.tensor.matmul(u[:], lhsT=s0Tm[:], rhs=Cv[:], start=True, stop=True)
                Rv = rc.tile([N1, 1], F32, tag="r")
                nc.vector.reciprocal(out=Rv[:], in_=u[:])
                w = mv.tile([N1, 1], F32, tag="mv")
                nc.tensor.matmul(w[:], lhsT=s0m[:], rhs=Rv[:], start=True, stop=True)
                Cv = rc.tile([N1, 1], F32, tag="c")
                nc.vector.reciprocal(out=Cv[:], in_=w[:])

        # S' = diag(R) S0; b2c = B2 scaled by C on the k (partition) axis
        sp = sb.tile([N1, N1], F32)
        nc.vector.tensor_scalar_mul(sp[:], s0[:], Rv[:])
        b2c = sb.tile([N1, L, N1], F32)
        nc.vector.tensor_scalar_mul(b2c[:], b2all, Cv[:])

        # Zt[k,(q,i)] = sum_j S'[j,k] P_q[j,i]   (one wide matmul)
        # F[i,l]     = sum_q sum_k Zt[k,(q,i)] C[k] B2_q[k,l]  (PSUM-accum)
        # Q          = sum_il F[i,l] S'[i,l] C[l]
        with tc.tile_pool(name="zt", bufs=1, space="PSUM") as ztp, \
                tc.tile_pool(name="fp", bufs=1, space="PSUM") as fpp, \
                tc.tile_pool(name="zsb", bufs=1) as zsb:
            zt_ps = ztp.tile([N1, L, N1], F32)
            nc.tensor.matmul(zt_ps[:].rearrange("p q i -> p (q i)"),
                             lhsT=sp[:], rhs=pmall, start=True, stop=True)
            zt = zsb.tile([N1, L, N1], F32)
            nc.vector.tensor_copy(out=zt[:], in_=zt_ps[:])

            f_ps = fpp.tile([N1, N1], F32)
            for q in range(L):
                nc.tensor.matmul(f_ps[:], lhsT=zt[:, q, :], rhs=b2c[:, q, :],
                                 start=(q == 0), stop=(q == L - 1))

            fs = sb.tile([N1, N1], F32)
            nc.vector.tensor_mul(fs[:], f_ps[:], sp[:])

        cs = sb.tile([N1, N1], F32)
        nc.vector.tensor_mul(cs[:], cg, sp[:])
        ds = sb.tile([N1, N1], F32)
        nc.vector.tensor_mul(ds[:], sp[:], sp[:])
        nc.vector.tensor_mul(ds[:], ds[:], dd)

        with tc.tile_pool(name="red", bufs=2, space="PSUM") as red, \
                tc.tile_pool(name="cols", bufs=1) as cols:
            q_ps = red.tile([N1, 1], F32, tag="red")
            nc.tensor.matmul(q_ps[:], lhsT=fs[:], rhs=ones_col[:], start=True, stop=True)
            qcol = cols.tile([N1, 1], F32)
            nc.vector.tensor_mul(qcol[:], q_ps[:], Cv[:])

            c_ps = red.tile([N1, 1], F32, tag="red")
            nc.tensor.matmul(c_ps[:], lhsT=cs[:], rhs=ones_col[:], start=True, stop=True)
            ccol = cols.tile([N1, 1], F32)
            nc.vector.tensor_mul(ccol[:], c_ps[:], Cv[:])

            d_ps = red.tile([N1, 1], F32, tag="red")
            nc.tensor.matmul(d_ps[:], lhsT=ds[:], rhs=ones_col[:], start=True, stop=True)
            dcol = cols.tile([N1, 1], F32)
            nc.vector.tensor_mul(dcol[:], d_ps[:], Cv[:])
            nc.vector.tensor_mul(dcol[:], dcol[:], Cv[:])

            comb = cols.tile([N1, 1], F32)
            nc.vector.scalar_tensor_tensor(out=comb[:], in0=qcol[:], scalar=0.5,
                                           in1=ccol[:], op0=mult, op1=add)
            nc.vector.scalar_tensor_tensor(out=comb[:], in0=dcol[:], scalar=-0.5,
                                           in1=comb[:], op0=mult, op1=add)

            tot_ps = red.tile([1, 1], F32, tag="tot")
            nc.tensor.matmul(tot_ps[:], lhsT=comb[:], rhs=ones_col[:],
                             start=True, stop=True)
            out_sb = cols.tile([1, 1], F32)
            nc.vector.tensor_copy(out=out_sb[:], in_=tot_ps[:])
            nc.sync.dma_start(out=out_d[:], in_=out_sb[:])

    if legalize:
        _legalize_waits(nc)
    return nc


def _host_prep(node_weights, edge_weights, A_g1, A_g2, labels1, labels2, n, m):
    n = int(n)
    m = int(m)
    n1, m1 = n + 1, m + 1
    assert n1 == N1 and m1 == N1, (n, m)

    cn = np.maximum(np.asarray(node_weights, np.float32), 0)
    ce = np.maximum(np.asarray(edge_weights, np.float32), 0)
    node_ins_del = cn[-1]
    edge_ins_del = ce[-1]
    node_costs = np.zeros((NB_LABELS, NB_LABELS), np.float32)
    node_costs[np.triu_indices(NB_LABELS, 1)] = cn[:-1]
    node_costs = node_costs + node_costs.T
    edge_costs = np.zeros((NB_EDGE_LABELS, NB_EDGE_LABELS), np.float32)
    edge_costs[np.triu_indices(NB_EDGE_LABELS, 1)] = ce[:-1]
    edge_costs = edge_costs + edge_costs.T

    A1 = np.zeros((n1, n1), np.int32)
    A1[:n, :n] = np.asarray(A_g1)[:n * n].reshape(n, n)
    A2 = np.zeros((m1, m1), np.int32)
    A2[:m, :m] = np.asarray(A_g2)[:m * m].reshape(m, m)

    T = np.zeros((L, L), np.float32)
    for a1 in range(L):
        for a2 in range(L):
            v = np.float32(0.0)
            if (a1 != 0) != (a2 != 0):
                v += edge_ins_del
            if a1 >= 1 and a2 >= 1:
                v += edge_costs[a1 - 1, a2 - 1]
            T[a1, a2] = v

    b2 = np.empty((m1, L, m1), np.float32)           # [k,q,l]
    for q in range(L):
        b2[:, q, :] = (A2 == q)
    TA1 = T[A1]                                       # [i,j,q]
    pmat = np.ascontiguousarray(TA1.transpose(1, 2, 0))  # [j,q,i]

    Dnm = node_costs[np.asarray(labels1)[:n][:, None], np.asarray(labels2)[:m][None, :]]
    cgrid = np.full((n1, m1), node_ins_del, np.float32)
    cgrid[:n, :m] = Dnm
    cgrid[n, m] = 0.0

    ddiag = T[A1.diagonal()[:, None], A2.diagonal()[None, :]].astype(np.float32)

    BIG = np.float32(1e4)
    cgmod = cgrid.copy()
    cgmod[:, m1 - 1] = BIG
    cgmod[n1 - 1, m1 - 1] = 0.0
    cgTmod = np.ascontiguousarray(cgrid.T)
    cgTmod[:, n1 - 1] = BIG
    cgTmod[m1 - 1, n1 - 1] = 0.0

    grids = np.stack([cgrid, cgmod, cgTmod, ddiag], axis=1)  # [96, 4, 96]
    tabs = np.stack([pmat, b2], axis=1)                      # [96, 2, L, 96]

    return {
        "grids": np.ascontiguousarray(grids),
        "tabs": np.ascontiguousarray(tabs),
    }


def run(inputs, trace=False, **spmd_kwargs):
    in_map = _host_prep(**inputs)
    if "nc" not in _NC_CACHE:
        _NC_CACHE["nc"] = _build_nc()
    nc = _NC_CACHE["nc"]
    core_ids = list(range(N_CORES))
    res = run_bass_kernel_spmd(
        nc, [dict(in_map) for _ in core_ids], core_ids, trace=trace, **spmd_kwargs
    )
    val = np.float32(res.results[0]["out"].reshape(()))
    return val, res


def kernel(**inputs) -> np.ndarray:
    val, _ = run(inputs)
    return np.asarray(val, np.float32).reshape(())



# revision 5
# speedup vs baseline: 1.1657x; 1.1657x over previous
"""Trainium2 Bass kernel for nn_GedLayer (graph edit distance forward).

The reference builds a 9216x9216 cost matrix C whose entries are a 4x4
lookup T[A1[i,j], A2[k,l]] over edge-label pairs, then computes
    ged = 0.5 * v @ (Dmat @ v) + c @ v
with v = vec(S) from a 10-iteration Sinkhorn on the 96x96 node-cost grid.

Because edge labels take only 4 values, the quadratic form factorizes into
96x96 matmuls (no 9216^2 matrix is ever formed):
    Zt[k,(q,i)] = sum_j S'[j,k] P_q[j,i]          one wide 96x96x384 matmul
    F[i,l]      = sum_qk Zt[k,(q,i)] C[k] B2_q[k,l]   4 PSUM-accum matmuls
    ged         = sum_m Cv[m]*colsum(G)[m] - 0.5*Cv[m]^2*colsum(H)[m]
with G = (0.5*F + cgrid) .* S', H = S'.^2 .* ddiag, S' = diag(R) S0, and
(R, C) from Sinkhorn run in vector form (R = 1/(S0m' C), C = 1/(S0Tm' R);
the "last scale pinned to 1" rule is implemented by baking an e_95 column
into the matvec operands so a full-tile reciprocal preserves the pin).

All device data is bf16 (PSUM accumulation stays fp32): a full-bf16
simulation of this pipeline vs the f64 oracle gives rel err ~3e-4, far
inside the 2e-2 gate, and bf16 halves DMA bytes and avoids the fp32
LOW_HIGH two-pass matmul emulation that doubles every LDWEIGHTS+MATMUL.

Sharding: one graph pair, strictly serial Sinkhorn recursion -> the
problem is latency-bound at 96x96 scale, so the computation is replicated
on all 8 cores (SPMD) and core 0's output is returned.
"""

import numpy as np
import ml_dtypes
from contextlib import ExitStack

import concourse.bass as bass
import concourse.tile as tile
from concourse import mybir
from concourse.bass_utils import run_bass_kernel_spmd

NB_LABELS = 10
NB_EDGE_LABELS = 3
SINKHORN_ITERS = 10
L = NB_EDGE_LABELS + 1
N1 = 96
F32 = mybir.dt.float32
BF16 = mybir.dt.bfloat16
N_CORES = 8

_NC_CACHE = {}


def _legalize_waits(nc):
    """Split multi-sem waits into standalone EventSemaphore instructions
    (this walrus codegen fits one sync wait per lowered instruction)."""
    n = 0
    for f in nc.m.functions:
        for bb in f.blocks:
            out = []
            for ins in bb.instructions:
                si = ins.sync_info
                waits = list(si.on_wait) if (si and si.on_wait) else []
                if len(waits) > 1:
                    for w in waits[:-1]:
                        n += 1
                        out.append(mybir.InstEventSemaphore(
                            name=f"LW-{n}",
                            engine=ins.engine,
                            ins=[],
                            outs=[],
                            sync_info=mybir.SyncInfo(on_wait=[w], on_update=[]),
                        ))
                    si.on_wait = [waits[-1]]
                out.append(ins)
            bb.instructions = out
    return n


def _build_nc(legalize=True):
    nc = bass.Bass()
    # crit = [cgTmod | cgmod] -- the two Sinkhorn matvec grids, DMA'd first
    crit_d = nc.dram_tensor("crit", [N1, 2, N1], BF16, kind="ExternalInput")
    # g2 = [cgrid | ddiag]
    g2_d = nc.dram_tensor("g2", [N1, 2, N1], BF16, kind="ExternalInput")
    pm_d = nc.dram_tensor("pm", [N1, L, N1], BF16, kind="ExternalInput")
    b2_d = nc.dram_tensor("b2", [N1, L, N1], BF16, kind="ExternalInput")
    out_d = nc.dram_tensor("out", [1, 1], F32, kind="ExternalOutput")

    ExpF = mybir.ActivationFunctionType.Exp
    CopyF = mybir.ActivationFunctionType.Copy
    mult = mybir.AluOpType.mult
    add = mybir.AluOpType.add

    with tile.TileContext(nc) as tc, ExitStack() as ctx, \
            nc.allow_low_precision("bf16 pipeline validated at 3e-4 rel err"):
        sb = ctx.enter_context(tc.tile_pool(name="sb", bufs=1))

        # Two parallel DMA queues (sync + gpsimd), Sinkhorn-critical first;
        # scalar stays free so the act-table load starts immediately.
        crit = sb.tile([N1, 2, N1], BF16)
        nc.sync.dma_start(out=crit[:], in_=crit_d[:])
        g2 = sb.tile([N1, 2, N1], BF16)
        nc.gpsimd.dma_start(out=g2[:], in_=g2_d[:])
        pm = sb.tile([N1, L, N1], BF16)
        nc.sync.dma_start(out=pm[:], in_=pm_d[:])
        b2 = sb.tile([N1, L, N1], BF16)
        nc.gpsimd.dma_start(out=b2[:], in_=b2_d[:])

        ones_bf = sb.tile([N1, 1], BF16)
        nc.vector.memset(ones_bf[:], 1.0)

        # exps in criticality order: s0Tm feeds the first matvec.
        s0Tm = sb.tile([N1, N1], BF16)
        nc.scalar.activation(out=s0Tm[:], in_=crit[:, 0, :], func=ExpF, scale=-0.5)
        s0m = sb.tile([N1, N1], BF16)
        nc.scalar.activation(out=s0m[:], in_=crit[:, 1, :], func=ExpF, scale=-0.5)
        s0 = sb.tile([N1, N1], BF16)
        nc.scalar.activation(out=s0[:], in_=g2[:, 0, :], func=ExpF, scale=-0.5)

        # Sinkhorn: fresh R/C tiles per iteration (no WAR deps -> each
        # matvec and reciprocal carries exactly one semaphore wait).
        Cv = ones_bf
        Rvf = Cvf = None
        with tc.tile_pool(name="mv", bufs=4, space="PSUM") as mv:
            for it in range(SINKHORN_ITERS):
                last = it == SINKHORN_ITERS - 1
                u = mv.tile([N1, 1], F32, tag="mv")
                nc.tensor.matmul(u[:], lhsT=s0Tm[:], rhs=Cv[:], start=True, stop=True)
                Rv = sb.tile([N1, 1], BF16)
                nc.vector.reciprocal(out=Rv[:], in_=u[:])
                if last:
                    # f32 twin for use as tensor_scalar/activation-scale
                    # operands (those require f32); runs off the chain.
                    Rvf = sb.tile([N1, 1], F32)
                    nc.vector.reciprocal(out=Rvf[:], in_=u[:])
                w = mv.tile([N1, 1], F32, tag="mv")
                nc.tensor.matmul(w[:], lhsT=s0m[:], rhs=Rv[:], start=True, stop=True)
                if last:
                    Cvf = sb.tile([N1, 1], F32)
                    nc.vector.reciprocal(out=Cvf[:], in_=w[:])
                else:
                    Cv = sb.tile([N1, 1], BF16)
                    nc.vector.reciprocal(out=Cv[:], in_=w[:])

        # S' = diag(R) S0 on the scalar engine (Copy shares Exp's table set);
        # b2c/H on gpsimd -- all three overlap the Sinkhorn tail / Zt matmul.
        sp = sb.tile([N1, N1], BF16)
        nc.scalar.activation(out=sp[:], in_=s0[:], func=CopyF, scale=Rvf[:])
        b2c = sb.tile([N1, L, N1], BF16)
        nc.gpsimd.tensor_scalar_mul(b2c[:], b2[:], Cvf[:])
        h1 = sb.tile([N1, N1], BF16)
        nc.gpsimd.tensor_mul(h1[:], sp[:], sp[:])
        H = sb.tile([N1, N1], BF16)
        nc.gpsimd.tensor_mul(H[:], h1[:], g2[:, 1, :])

        # Zt[k,(q,i)] = sum_j S'[j,k] P_q[j,i]   (one wide matmul)
        # F[i,l]     = sum_q sum_k Zt[k,(q,i)] C[k] B2_q[k,l]  (PSUM-accum)
        with tc.tile_pool(name="zt", bufs=1, space="PSUM") as ztp, \
                tc.tile_pool(name="fp", bufs=1, space="PSUM") as fpp, \
                tc.tile_pool(name="red", bufs=3, space="PSUM") as red:
            zt_ps = ztp.tile([N1, L, N1], F32)
            nc.tensor.matmul(zt_ps[:].rearrange("p q i -> p (q i)"),
                             lhsT=sp[:], rhs=pm[:].rearrange("p q i -> p (q i)"),
                             start=True, stop=True)
            zt = sb.tile([N1, L, N1], BF16)
            nc.vector.tensor_copy(out=zt[:, 0:2, :], in_=zt_ps[:, 0:2, :])
            nc.scalar.activation(out=zt[:, 2:4, :], in_=zt_ps[:, 2:4, :], func=CopyF)

            f_ps = fpp.tile([N1, N1], F32)
            for q in range(L):
                nc.tensor.matmul(f_ps[:], lhsT=zt[:, q, :], rhs=b2c[:, q, :],
                                 start=(q == 0), stop=(q == L - 1))

            # G = (0.5 F + cgrid) .* S'
            g1 = sb.tile([N1, N1], BF16)
            nc.vector.scalar_tensor_tensor(out=g1[:], in0=f_ps[:], scalar=0.5,
                                           in1=g2[:, 0, :], op0=mult, op1=add)
            G = sb.tile([N1, N1], BF16)
            nc.vector.tensor_mul(G[:], g1[:], sp[:])

            q_ps = red.tile([N1, 1], F32, tag="red")
            nc.tensor.matmul(q_ps[:], lhsT=G[:], rhs=ones_bf[:], start=True, stop=True)
            h_ps = red.tile([N1, 1], F32, tag="red")
            nc.tensor.matmul(h_ps[:], lhsT=H[:], rhs=ones_bf[:], start=True, stop=True)

            # ged = sum_m Cv[m]*(q[m] - 0.5*Cv[m]*h[m])
            t1 = sb.tile([N1, 1], BF16)
            nc.vector.tensor_mul(t1[:], h_ps[:], Cvf[:])
            t2 = sb.tile([N1, 1], BF16)
            nc.vector.scalar_tensor_tensor(out=t2[:], in0=t1[:], scalar=-0.5,
                                           in1=q_ps[:], op0=mult, op1=add)
            t3 = sb.tile([N1, 1], BF16)
            nc.vector.tensor_mul(t3[:], t2[:], Cvf[:])

            tot_ps = red.tile([1, 1], F32, tag="tot")
            nc.tensor.matmul(tot_ps[:], lhsT=t3[:], rhs=ones_bf[:],
                             start=True, stop=True)
            out_sb = sb.tile([1, 1], F32)
            nc.vector.tensor_copy(out=out_sb[:], in_=tot_ps[:])
            nc.sync.dma_start(out=out_d[:], in_=out_sb[:])

    if legalize:
        _legalize_waits(nc)
    return nc


def _host_prep(node_weights, edge_weights, A_g1, A_g2, labels1, labels2, n, m):
    n = int(n)
    m = int(m)
    n1, m1 = n + 1, m + 1
    assert n1 == N1 and m1 == N1, (n, m)

    cn = np.maximum(np.asarray(node_weights, np.float32), 0)
    ce = np.maximum(np.asarray(edge_weights, np.float32), 0)
    node_ins_del = cn[-1]
    edge_ins_del = ce[-1]
    node_costs = np.zeros((NB_LABELS, NB_LABELS), np.float32)
    node_costs[np.triu_indices(NB_LABELS, 1)] = cn[:-1]
    node_costs = node_costs + node_costs.T
    edge_costs = np.zeros((NB_EDGE_LABELS, NB_EDGE_LABELS), np.float32)
    edge_costs[np.triu_indices(NB_EDGE_LABELS, 1)] = ce[:-1]
    edge_costs = edge_costs + edge_costs.T

    A1 = np.zeros((n1, n1), np.int32)
    A1[:n, :n] = np.asarray(A_g1)[:n * n].reshape(n, n)
    A2 = np.zeros((m1, m1), np.int32)
    A2[:m, :m] = np.asarray(A_g2)[:m * m].reshape(m, m)

    T = np.zeros((L, L), np.float32)
    for a1 in range(L):
        for a2 in range(L):
            v = np.float32(0.0)
            if (a1 != 0) != (a2 != 0):
                v += edge_ins_del
            if a1 >= 1 and a2 >= 1:
                v += edge_costs[a1 - 1, a2 - 1]
            T[a1, a2] = v

    b2 = np.empty((m1, L, m1), np.float32)           # [k,q,l]
    for q in range(L):
        b2[:, q, :] = (A2 == q)
    TA1 = T[A1]                                       # [i,j,q]
    pmat = np.ascontiguousarray(TA1.transpose(1, 2, 0))  # [j,q,i]

    Dnm = node_costs[np.asarray(labels1)[:n][:, None], np.asarray(labels2)[:m][None, :]]
    cgrid = np.full((n1, m1), node_ins_del, np.float32)
    cgrid[:n, :m] = Dnm
    cgrid[n, m] = 0.0

    ddiag = T[A1.diagonal()[:, None], A2.diagonal()[None, :]].astype(np.float32)

    BIG = np.float32(1e4)
    cgmod = cgrid.copy()
    cgmod[:, m1 - 1] = BIG
    cgmod[n1 - 1, m1 - 1] = 0.0
    cgTmod = np.ascontiguousarray(cgrid.T)
    cgTmod[:, n1 - 1] = BIG
    cgTmod[m1 - 1, n1 - 1] = 0.0

    bf = ml_dtypes.bfloat16
    crit = np.stack([cgTmod, cgmod], axis=1).astype(bf)   # [96, 2, 96]
    g2 = np.stack([cgrid, ddiag], axis=1).astype(bf)      # [96, 2, 96]

    return {
        "crit": np.ascontiguousarray(crit),
        "g2": np.ascontiguousarray(g2),
        "pm": np.ascontiguousarray(pmat.astype(bf)),
        "b2": np.ascontiguousarray(b2.astype(bf)),
    }


def run(inputs, trace=False, **spmd_kwargs):
    in_map = _host_prep(**inputs)
    if "nc" not in _NC_CACHE:
        _NC_CACHE["nc"] = _build_nc()
    nc = _NC_CACHE["nc"]
    core_ids = list(range(N_CORES))
    res = run_bass_kernel_spmd(
        nc, [dict(in_map) for _ in core_ids], core_ids, trace=trace, **spmd_kwargs
    )
    val = np.float32(res.results[0]["out"].reshape(()))
    return val, res


def kernel(**inputs) -> np.ndarray:
    val, _ = run(inputs)
    return np.asarray(val, np.float32).reshape(())


# revision 8
# speedup vs baseline: 1.3555x; 1.1628x over previous
"""Trainium2 Bass kernel for nn_GedLayer (graph edit distance forward).

The reference builds a 9216x9216 cost matrix C whose entries are a 4x4
lookup T[A1[i,j], A2[k,l]] over edge-label pairs, then computes
    ged = 0.5 * v @ (Dmat @ v) + c @ v
with v = vec(S) from a 10-iteration Sinkhorn on the 96x96 node-cost grid.

Because edge labels take only 4 values, the quadratic form factorizes into
96x96 matmuls (no 9216^2 matrix is ever formed):
    Zt[k,(q,i)] = sum_j S'[j,k] P_q[j,i]          one wide 96x96x384 matmul
    F[i,l]      = sum_qk Zt[k,(q,i)] C[k] B2_q[k,l]   4 PSUM-accum matmuls
    ged         = sum_m Cv[m]*colsum(G)[m] - 0.5*Cv[m]^2*colsum(H)[m]
with G = (0.5*F + cgrid) .* S', H = S'.^2 .* ddiag, S' = diag(R) S0, and
(R, C) from Sinkhorn run in vector form (R = 1/(S0m' C), C = 1/(S0Tm' R);
the "last scale pinned to 1" rule is implemented by baking an e_95 column
into the matvec operands so a full-tile reciprocal preserves the pin).

All device data is bf16 (PSUM accumulation stays fp32): a full-bf16
simulation of this pipeline vs the f64 oracle gives rel err ~3e-4, far
inside the 2e-2 gate. bf16 halves DMA bytes and avoids the fp32 LOW_HIGH
two-pass matmul emulation that doubles every LDWEIGHTS+MATMUL. The host
ships exp(-c/2) directly (bit-equivalent to exp-on-device at bf16) so no
activation table load or serial EXPs sit on the critical path, and the
first DMA is split across two queues because small-row DMAs here are
descriptor-rate-bound (~27ns/row), not bandwidth-bound.

Sharding: one graph pair, strictly serial Sinkhorn recursion -> the
problem is latency-bound at 96x96 scale, so the computation is replicated
on all 8 cores (SPMD) and core 0's output is returned.
"""

import numpy as np
import ml_dtypes
from contextlib import ExitStack

import concourse.bass as bass
import concourse.tile as tile
from concourse import mybir
from concourse.bass_utils import run_bass_kernel_spmd

NB_LABELS = 10
NB_EDGE_LABELS = 3
SINKHORN_ITERS = 10
L = NB_EDGE_LABELS + 1
N1 = 96
F32 = mybir.dt.float32
BF16 = mybir.dt.bfloat16
N_CORES = 8

_NC_CACHE = {}


def _legalize_waits(nc):
    """Split multi-sem waits into standalone EventSemaphore instructions
    (this walrus codegen fits one sync wait per lowered instruction)."""
    n = 0
    for f in nc.m.functions:
        for bb in f.blocks:
            out = []
            for ins in bb.instructions:
                si = ins.sync_info
                waits = list(si.on_wait) if (si and si.on_wait) else []
                if len(waits) > 1:
                    for w in waits[:-1]:
                        n += 1
                        out.append(mybir.InstEventSemaphore(
                            name=f"LW-{n}",
                            engine=ins.engine,
                            ins=[],
                            outs=[],
                            sync_info=mybir.SyncInfo(on_wait=[w], on_update=[]),
                        ))
                    si.on_wait = [waits[-1]]
                out.append(ins)
            bb.instructions = out
    return n


def _build_nc(legalize=True):
    nc = bass.Bass()
    # crit = [s0Tm | s0m] -- the Sinkhorn matvec operands, exp'd on host.
    crit_d = nc.dram_tensor("crit", [N1, 2, N1], BF16, kind="ExternalInput")
    # g2 = [s0 | ddiag | cgrid]
    g2_d = nc.dram_tensor("g2", [N1, 3, N1], BF16, kind="ExternalInput")
    pm_d = nc.dram_tensor("pm", [N1, L, N1], BF16, kind="ExternalInput")
    b2_d = nc.dram_tensor("b2", [N1, L, N1], BF16, kind="ExternalInput")
    out_d = nc.dram_tensor("out", [1, 1], F32, kind="ExternalOutput")

    mult = mybir.AluOpType.mult
    add = mybir.AluOpType.add

    with tile.TileContext(nc) as tc, ExitStack() as ctx, \
            nc.allow_low_precision("bf16 pipeline validated at 3e-4 rel err"):
        sb = ctx.enter_context(tc.tile_pool(name="sb", bufs=1))

        # crit row-split across both DMA queues (descriptor-rate-bound);
        # the bulk tensors follow behind on the same queues.
        crit = sb.tile([N1, 2, N1], BF16)
        H2 = N1 // 2
        nc.sync.dma_start(out=crit[0:H2], in_=crit_d[0:H2])
        nc.gpsimd.dma_start(out=crit[H2:N1], in_=crit_d[H2:N1])
        pm = sb.tile([N1, L, N1], BF16)
        nc.sync.dma_start(out=pm[:], in_=pm_d[:])
        g2 = sb.tile([N1, 3, N1], BF16)
        nc.gpsimd.dma_start(out=g2[:], in_=g2_d[:])
        b2 = sb.tile([N1, L, N1], BF16)
        nc.sync.dma_start(out=b2[:], in_=b2_d[:])

        ones_bf = sb.tile([N1, 1], BF16)
        nc.vector.memset(ones_bf[:], 1.0)

        s0Tm = crit[:, 0, :]
        s0m = crit[:, 1, :]
        s0 = g2[:, 0, :]
        dd = g2[:, 1, :]
        cg = g2[:, 2, :]

        # Sinkhorn: fresh R/C tiles per iteration (no WAR deps -> each
        # matvec and reciprocal carries exactly one semaphore wait).
        Cv = ones_bf
        Rvf = Cvf = None
        with tc.tile_pool(name="mv", bufs=4, space="PSUM") as mv:
            for it in range(SINKHORN_ITERS):
                last = it == SINKHORN_ITERS - 1
                u = mv.tile([N1, 1], F32, tag="mv")
                nc.tensor.matmul(u[:], lhsT=s0Tm, rhs=Cv[:], start=True, stop=True)
                Rv = sb.tile([N1, 1], BF16)
                nc.vector.reciprocal(out=Rv[:], in_=u[:])
                if last:
                    # f32 twin for use as tensor_scalar operands (those
                    # require f32 scalars); runs off the chain.
                    Rvf = sb.tile([N1, 1], F32)
                    nc.vector.reciprocal(out=Rvf[:], in_=u[:])
                w = mv.tile([N1, 1], F32, tag="mv")
                nc.tensor.matmul(w[:], lhsT=s0m, rhs=Rv[:], start=True, stop=True)
                if last:
                    Cvf = sb.tile([N1, 1], F32)
                    nc.vector.reciprocal(out=Cvf[:], in_=w[:])
                else:
                    Cv = sb.tile([N1, 1], BF16)
                    nc.vector.reciprocal(out=Cv[:], in_=w[:])

        # Post-Sinkhorn scalings on the vector engine (gpsimd tensor_scalar
        # is ~15x slower); G1/H products on gpsimd, off the critical path.
        sp = sb.tile([N1, N1], BF16)
        nc.vector.tensor_scalar_mul(sp[:], s0, Rvf[:])
        b2c = sb.tile([N1, L, N1], BF16)
        nc.vector.tensor_scalar_mul(b2c[:], b2[:], Cvf[:])
        nhc2 = sb.tile([N1, 1], F32)  # -0.5 * Cv^2
        nc.vector.tensor_scalar(nhc2[:], Cvf[:], Cvf[:], -0.5, op0=mult, op1=mult)

        G1 = sb.tile([N1, N1], BF16)  # cgrid .* S'
        nc.gpsimd.tensor_mul(G1[:], cg, sp[:])
        h1 = sb.tile([N1, N1], BF16)
        nc.gpsimd.tensor_mul(h1[:], sp[:], sp[:])
        H = sb.tile([N1, N1], BF16)  # S'.^2 .* ddiag
        nc.gpsimd.tensor_mul(H[:], h1[:], dd)

        with tc.tile_pool(name="zt", bufs=1, space="PSUM") as ztp, \
                tc.tile_pool(name="fp", bufs=1, space="PSUM") as fpp, \
                tc.tile_pool(name="red", bufs=1, space="PSUM") as red:
            # Zt[k,(q,i)] = sum_j S'[j,k] P_q[j,i]
            zt_ps = ztp.tile([N1, L, N1], F32)
            nc.tensor.matmul(zt_ps[:].rearrange("p q i -> p (q i)"),
                             lhsT=sp[:], rhs=pm[:].rearrange("p q i -> p (q i)"),
                             start=True, stop=True)
            # colsum(G1) accumulates first into q_ps; G2's colsum lands on
            # top after F is ready (interleaved PSUM groups, distinct banks)
            q_ps = red.tile([N1, 1], F32, tag="q")
            nc.tensor.matmul(q_ps[:], lhsT=G1[:], rhs=ones_bf[:],
                             start=True, stop=False, skip_group_check=True)

            # PSUM -> SBUF for the F matmuls, split across two engines; two
            # tiles so F's q=0,1 don't wait on the scalar half.
            zt01 = sb.tile([N1, 2, N1], BF16)
            nc.vector.tensor_copy(out=zt01[:], in_=zt_ps[:, 0:2, :])
            zt23 = sb.tile([N1, 2, N1], BF16)
            nc.scalar.activation(out=zt23[:], in_=zt_ps[:, 2:4, :],
                                 func=mybir.ActivationFunctionType.Copy)

            f_ps = fpp.tile([N1, N1], F32)
            for q in range(L):
                zt_q = (zt01 if q < 2 else zt23)[:, q % 2, :]
                nc.tensor.matmul(f_ps[:], lhsT=zt_q, rhs=b2c[:, q, :],
                                 start=(q == 0), stop=(q == L - 1),
                                 skip_group_check=True)

            h_ps = red.tile([N1, 1], F32, tag="h")
            nc.tensor.matmul(h_ps[:], lhsT=H[:], rhs=ones_bf[:],
                             start=True, stop=True, skip_group_check=True)
            # v = colsum(H) .* (-0.5 Cv^2)  (ready before q finishes)
            v = sb.tile([N1, 1], BF16)
            nc.vector.tensor_mul(v[:], h_ps[:], nhc2[:])

            # G2 = (0.5 F) .* S' in one fused op, then its colsum
            G2 = sb.tile([N1, N1], BF16)
            nc.vector.scalar_tensor_tensor(out=G2[:], in0=f_ps[:], scalar=0.5,
                                           in1=sp[:], op0=mult, op1=mult)
            nc.tensor.matmul(q_ps[:], lhsT=G2[:], rhs=ones_bf[:],
                             start=False, stop=True, skip_group_check=True)
            wv = sb.tile([N1, 1], BF16)
            nc.vector.tensor_mul(wv[:], q_ps[:], Cvf[:])

            # ged = sum(v) + sum(wv), accumulated on the PE
            tot_ps = red.tile([1, 1], F32, tag="tot")
            nc.tensor.matmul(tot_ps[:], lhsT=v[:], rhs=ones_bf[:],
                             start=True, stop=False, skip_group_check=True)
            nc.tensor.matmul(tot_ps[:], lhsT=wv[:], rhs=ones_bf[:],
                             start=False, stop=True, skip_group_check=True)
            out_sb = sb.tile([1, 1], F32)
            nc.vector.tensor_copy(out=out_sb[:], in_=tot_ps[:])
            nc.sync.dma_start(out=out_d[:], in_=out_sb[:])

    if legalize:
        _legalize_waits(nc)
    return nc


def _host_prep(node_weights, edge_weights, A_g1, A_g2, labels1, labels2, n, m):
    n = int(n)
    m = int(m)
    n1, m1 = n + 1, m + 1
    assert n1 == N1 and m1 == N1, (n, m)

    cn = np.maximum(np.asarray(node_weights, np.float32), 0)
    ce = np.maximum(np.asarray(edge_weights, np.float32), 0)
    node_ins_del = cn[-1]
    edge_ins_del = ce[-1]
    node_costs = np.zeros((NB_LABELS, NB_LABELS), np.float32)
    node_costs[np.triu_indices(NB_LABELS, 1)] = cn[:-1]
    node_costs = node_costs + node_costs.T
    edge_costs = np.zeros((NB_EDGE_LABELS, NB_EDGE_LABELS), np.float32)
    edge_costs[np.triu_indices(NB_EDGE_LABELS, 1)] = ce[:-1]
    edge_costs = edge_costs + edge_costs.T

    A1 = np.zeros((n1, n1), np.int32)
    A1[:n, :n] = np.asarray(A_g1)[:n * n].reshape(n, n)
    A2 = np.zeros((m1, m1), np.int32)
    A2[:m, :m] = np.asarray(A_g2)[:m * m].reshape(m, m)

    T = np.zeros((L, L), np.float32)
    for a1 in range(L):
        for a2 in range(L):
            v = np.float32(0.0)
            if (a1 != 0) != (a2 != 0):
                v += edge_ins_del
            if a1 >= 1 and a2 >= 1:
                v += edge_costs[a1 - 1, a2 - 1]
            T[a1, a2] = v

    b2 = np.empty((m1, L, m1), np.float32)           # [k,q,l]
    for q in range(L):
        b2[:, q, :] = (A2 == q)
    TA1 = T[A1]                                       # [i,j,q]
    pmat = np.ascontiguousarray(TA1.transpose(1, 2, 0))  # [j,q,i]

    Dnm = node_costs[np.asarray(labels1)[:n][:, None], np.asarray(labels2)[:m][None, :]]
    cgrid = np.full((n1, m1), node_ins_del, np.float32)
    cgrid[:n, :m] = Dnm
    cgrid[n, m] = 0.0

    ddiag = T[A1.diagonal()[:, None], A2.diagonal()[None, :]].astype(np.float32)

    BIG = np.float32(1e4)
    cgmod = cgrid.copy()
    cgmod[:, m1 - 1] = BIG
    cgmod[n1 - 1, m1 - 1] = 0.0
    cgTmod = np.ascontiguousarray(cgrid.T)
    cgTmod[:, n1 - 1] = BIG
    cgTmod[m1 - 1, n1 - 1] = 0.0

    bf = ml_dtypes.bfloat16
    s0Tm = np.exp(-0.5 * cgTmod.astype(np.float64)).astype(bf)
    s0m = np.exp(-0.5 * cgmod.astype(np.float64)).astype(bf)
    s0 = np.exp(-0.5 * cgrid.astype(np.float64)).astype(bf)
    crit = np.stack([s0Tm, s0m], axis=1)                        # [96, 2, 96]
    g2 = np.stack([s0, ddiag.astype(bf), cgrid.astype(bf)], axis=1)

    return {
        "crit": np.ascontiguousarray(crit),
        "g2": np.ascontiguousarray(g2),
        "pm": np.ascontiguousarray(pmat.astype(bf)),
        "b2": np.ascontiguousarray(b2.astype(bf)),
    }


def run(inputs, trace=False, **spmd_kwargs):
    in_map = _host_prep(**inputs)
    if "nc" not in _NC_CACHE:
        _NC_CACHE["nc"] = _build_nc()
    nc = _NC_CACHE["nc"]
    core_ids = list(range(N_CORES))
    res = run_bass_kernel_spmd(
        nc, [dict(in_map) for _ in core_ids], core_ids, trace=trace, **spmd_kwargs
    )
    val = np.float32(res.results[0]["out"].reshape(()))
    return val, res


def kernel(**inputs) -> np.ndarray:
    val, _ = run(inputs)
    return np.asarray(val, np.float32).reshape(())


# revision 9
# speedup vs baseline: 1.3758x; 1.0150x over previous
"""Trainium2 Bass kernel for nn_GedLayer (graph edit distance forward).

The reference builds a 9216x9216 cost matrix C whose entries are a 4x4
lookup T[A1[i,j], A2[k,l]] over edge-label pairs, then computes
    ged = 0.5 * v @ (Dmat @ v) + c @ v
with v = vec(S) from a 10-iteration Sinkhorn on the 96x96 node-cost grid.

Because edge labels take only 4 values, the quadratic form factorizes into
96x96 matmuls (no 9216^2 matrix is ever formed):
    Zt[k,(q,i)] = sum_j S'[j,k] P_q[j,i]          one wide 96x96x384 matmul
    F[i,l]      = sum_qk Zt[k,(q,i)] C[k] B2_q[k,l]   4 PSUM-accum matmuls
    ged         = sum_m Cv[m]*colsum(G)[m] - 0.5*Cv[m]^2*colsum(H)[m]
with G = (0.5*F + cgrid) .* S', H = S'.^2 .* ddiag, S' = diag(R) S0, and
(R, C) from Sinkhorn run in vector form (R = 1/(S0m' C), C = 1/(S0Tm' R);
the "last scale pinned to 1" rule is implemented by baking an e_95 column
into the matvec operands so a full-tile reciprocal preserves the pin).

All device data is bf16 (PSUM accumulation stays fp32): a full-bf16
simulation of this pipeline vs the f64 oracle gives rel err ~3e-4, far
inside the 2e-2 gate. bf16 halves DMA bytes and avoids the fp32 LOW_HIGH
two-pass matmul emulation that doubles every LDWEIGHTS+MATMUL. The host
ships exp(-c/2) directly (bit-equivalent to exp-on-device at bf16) so no
activation table load or serial EXPs sit on the critical path, and the
first DMA is split across two queues because small-row DMAs here are
descriptor-rate-bound (~27ns/row), not bandwidth-bound.

Sharding: one graph pair, strictly serial Sinkhorn recursion -> the
problem is latency-bound at 96x96 scale, so the computation is replicated
on all 8 cores (SPMD) and core 0's output is returned.
"""

import numpy as np
import ml_dtypes
from contextlib import ExitStack

import concourse.bass as bass
import concourse.tile as tile
from concourse import mybir
from concourse.bass_utils import run_bass_kernel_spmd

NB_LABELS = 10
NB_EDGE_LABELS = 3
SINKHORN_ITERS = 10
L = NB_EDGE_LABELS + 1
N1 = 96
F32 = mybir.dt.float32
BF16 = mybir.dt.bfloat16
N_CORES = 8

_NC_CACHE = {}


def _legalize_waits(nc):
    """Split multi-sem waits into standalone EventSemaphore instructions
    (this walrus codegen fits one sync wait per lowered instruction)."""
    n = 0
    for f in nc.m.functions:
        for bb in f.blocks:
            out = []
            for ins in bb.instructions:
                si = ins.sync_info
                waits = list(si.on_wait) if (si and si.on_wait) else []
                if len(waits) > 1:
                    for w in waits[:-1]:
                        n += 1
                        out.append(mybir.InstEventSemaphore(
                            name=f"LW-{n}",
                            engine=ins.engine,
                            ins=[],
                            outs=[],
                            sync_info=mybir.SyncInfo(on_wait=[w], on_update=[]),
                        ))
                    si.on_wait = [waits[-1]]
                out.append(ins)
            bb.instructions = out
    return n


def _build_nc(legalize=True):
    nc = bass.Bass()
    # crit = [s0Tm | s0m] -- the Sinkhorn matvec operands, exp'd on host.
    crit_d = nc.dram_tensor("crit", [N1, 2, N1], BF16, kind="ExternalInput")
    # g2 = [s0 | ddiag | cgrid]
    g2_d = nc.dram_tensor("g2", [N1, 3, N1], BF16, kind="ExternalInput")
    pm_d = nc.dram_tensor("pm", [N1, L, N1], BF16, kind="ExternalInput")
    b2_d = nc.dram_tensor("b2", [N1, L, N1], BF16, kind="ExternalInput")
    out_d = nc.dram_tensor("out", [1, 1], F32, kind="ExternalOutput")

    mult = mybir.AluOpType.mult
    add = mybir.AluOpType.add

    with tile.TileContext(nc) as tc, ExitStack() as ctx, \
            nc.allow_low_precision("bf16 pipeline validated at 3e-4 rel err"):
        sb = ctx.enter_context(tc.tile_pool(name="sb", bufs=1))

        # crit row-split across all three DMA queues (these DMAs are
        # descriptor-rate-bound, ~30ns/row, so rows not bytes matter);
        # the bulk tensors follow behind on the same queues.
        crit = sb.tile([N1, 2, N1], BF16)
        T3 = N1 // 3
        nc.sync.dma_start(out=crit[0:T3], in_=crit_d[0:T3])
        nc.scalar.dma_start(out=crit[T3:2 * T3], in_=crit_d[T3:2 * T3])
        nc.gpsimd.dma_start(out=crit[2 * T3:N1], in_=crit_d[2 * T3:N1])
        pm = sb.tile([N1, L, N1], BF16)
        nc.sync.dma_start(out=pm[:], in_=pm_d[:])
        g2 = sb.tile([N1, 3, N1], BF16)
        nc.gpsimd.dma_start(out=g2[:], in_=g2_d[:])
        b2 = sb.tile([N1, L, N1], BF16)
        nc.sync.dma_start(out=b2[:], in_=b2_d[:])

        ones_bf = sb.tile([N1, 1], BF16)
        nc.vector.memset(ones_bf[:], 1.0)

        # Dummy activation so walrus hoists the 1.3us activation-table load
        # here (overlapping the DMA wait) instead of before the epilogue's
        # PSUM->SBUF copy.
        dmy = sb.tile([1, 1], BF16)
        nc.scalar.activation(out=dmy[:], in_=ones_bf[0:1, :],
                             func=mybir.ActivationFunctionType.Copy)

        s0Tm = crit[:, 0, :]
        s0m = crit[:, 1, :]
        s0 = g2[:, 0, :]
        dd = g2[:, 1, :]
        cg = g2[:, 2, :]

        # Sinkhorn: fresh R/C tiles per iteration (no WAR deps -> each
        # matvec and reciprocal carries exactly one semaphore wait).
        Cv = ones_bf
        Rvf = Cvf = None
        with tc.tile_pool(name="mv", bufs=4, space="PSUM") as mv:
            for it in range(SINKHORN_ITERS):
                last = it == SINKHORN_ITERS - 1
                u = mv.tile([N1, 1], F32, tag="mv")
                nc.tensor.matmul(u[:], lhsT=s0Tm, rhs=Cv[:], start=True, stop=True)
                Rv = sb.tile([N1, 1], BF16)
                nc.vector.reciprocal(out=Rv[:], in_=u[:])
                if last:
                    # f32 twin for use as tensor_scalar operands (those
                    # require f32 scalars); runs off the chain.
                    Rvf = sb.tile([N1, 1], F32)
                    nc.vector.reciprocal(out=Rvf[:], in_=u[:])
                w = mv.tile([N1, 1], F32, tag="mv")
                nc.tensor.matmul(w[:], lhsT=s0m, rhs=Rv[:], start=True, stop=True)
                if last:
                    Cvf = sb.tile([N1, 1], F32)
                    nc.vector.reciprocal(out=Cvf[:], in_=w[:])
                else:
                    Cv = sb.tile([N1, 1], BF16)
                    nc.vector.reciprocal(out=Cv[:], in_=w[:])

        # Post-Sinkhorn scalings on the vector engine (gpsimd tensor_scalar
        # is ~15x slower); G1/H products on gpsimd, off the critical path.
        sp = sb.tile([N1, N1], BF16)
        nc.vector.tensor_scalar_mul(sp[:], s0, Rvf[:])
        b2c = sb.tile([N1, L, N1], BF16)
        nc.vector.tensor_scalar_mul(b2c[:], b2[:], Cvf[:])
        nhc2 = sb.tile([N1, 1], F32)  # -0.5 * Cv^2
        nc.vector.tensor_scalar(nhc2[:], Cvf[:], Cvf[:], -0.5, op0=mult, op1=mult)

        G1 = sb.tile([N1, N1], BF16)  # cgrid .* S'
        nc.gpsimd.tensor_mul(G1[:], cg, sp[:])
        h1 = sb.tile([N1, N1], BF16)
        nc.gpsimd.tensor_mul(h1[:], sp[:], sp[:])
        H = sb.tile([N1, N1], BF16)  # S'.^2 .* ddiag
        nc.gpsimd.tensor_mul(H[:], h1[:], dd)

        with tc.tile_pool(name="zt", bufs=1, space="PSUM") as ztp, \
                tc.tile_pool(name="fp", bufs=1, space="PSUM") as fpp, \
                tc.tile_pool(name="red", bufs=1, space="PSUM") as red:
            # Zt[k,(q,i)] = sum_j S'[j,k] P_q[j,i]
            zt_ps = ztp.tile([N1, L, N1], F32)
            nc.tensor.matmul(zt_ps[:].rearrange("p q i -> p (q i)"),
                             lhsT=sp[:], rhs=pm[:].rearrange("p q i -> p (q i)"),
                             start=True, stop=True)
            # colsum(G1) accumulates first into q_ps; G2's colsum lands on
            # top after F is ready (interleaved PSUM groups, distinct banks)
            q_ps = red.tile([N1, 1], F32, tag="q")
            nc.tensor.matmul(q_ps[:], lhsT=G1[:], rhs=ones_bf[:],
                             start=True, stop=False, skip_group_check=True)

            # PSUM -> SBUF for the F matmuls, split across two engines; two
            # tiles so F's q=0,1 don't wait on the scalar half.
            zt01 = sb.tile([N1, 2, N1], BF16)
            nc.vector.tensor_copy(out=zt01[:], in_=zt_ps[:, 0:2, :])
            zt23 = sb.tile([N1, 2, N1], BF16)
            nc.scalar.activation(out=zt23[:], in_=zt_ps[:, 2:4, :],
                                 func=mybir.ActivationFunctionType.Copy)

            f_ps = fpp.tile([N1, N1], F32)
            for q in range(L):
                zt_q = (zt01 if q < 2 else zt23)[:, q % 2, :]
                nc.tensor.matmul(f_ps[:], lhsT=zt_q, rhs=b2c[:, q, :],
                                 start=(q == 0), stop=(q == L - 1),
                                 skip_group_check=True)

            h_ps = red.tile([N1, 1], F32, tag="h")
            nc.tensor.matmul(h_ps[:], lhsT=H[:], rhs=ones_bf[:],
                             start=True, stop=True, skip_group_check=True)
            # v = colsum(H) .* (-0.5 Cv^2)  (ready before q finishes)
            v = sb.tile([N1, 1], BF16)
            nc.vector.tensor_mul(v[:], h_ps[:], nhc2[:])

            # G2 = (0.5 F) .* S' in one fused op, then its colsum
            G2 = sb.tile([N1, N1], BF16)
            nc.vector.scalar_tensor_tensor(out=G2[:], in0=f_ps[:], scalar=0.5,
                                           in1=sp[:], op0=mult, op1=mult)
            nc.tensor.matmul(q_ps[:], lhsT=G2[:], rhs=ones_bf[:],
                             start=False, stop=True, skip_group_check=True)
            wv = sb.tile([N1, 1], BF16)
            nc.vector.tensor_mul(wv[:], q_ps[:], Cvf[:])

            # ged = sum(v) + sum(wv), accumulated on the PE
            tot_ps = red.tile([1, 1], F32, tag="tot")
            nc.tensor.matmul(tot_ps[:], lhsT=v[:], rhs=ones_bf[:],
                             start=True, stop=False, skip_group_check=True)
            nc.tensor.matmul(tot_ps[:], lhsT=wv[:], rhs=ones_bf[:],
                             start=False, stop=True, skip_group_check=True)
            out_sb = sb.tile([1, 1], F32)
            nc.vector.tensor_copy(out=out_sb[:], in_=tot_ps[:])
            nc.sync.dma_start(out=out_d[:], in_=out_sb[:])

    if legalize:
        _legalize_waits(nc)
    return nc


def _host_prep(node_weights, edge_weights, A_g1, A_g2, labels1, labels2, n, m):
    n = int(n)
    m = int(m)
    n1, m1 = n + 1, m + 1
    assert n1 == N1 and m1 == N1, (n, m)

    cn = np.maximum(np.asarray(node_weights, np.float32), 0)
    ce = np.maximum(np.asarray(edge_weights, np.float32), 0)
    node_ins_del = cn[-1]
    edge_ins_del = ce[-1]
    node_costs = np.zeros((NB_LABELS, NB_LABELS), np.float32)
    node_costs[np.triu_indices(NB_LABELS, 1)] = cn[:-1]
    node_costs = node_costs + node_costs.T
    edge_costs = np.zeros((NB_EDGE_LABELS, NB_EDGE_LABELS), np.float32)
    edge_costs[np.triu_indices(NB_EDGE_LABELS, 1)] = ce[:-1]
    edge_costs = edge_costs + edge_costs.T

    A1 = np.zeros((n1, n1), np.int32)
    A1[:n, :n] = np.asarray(A_g1)[:n * n].reshape(n, n)
    A2 = np.zeros((m1, m1), np.int32)
    A2[:m, :m] = np.asarray(A_g2)[:m * m].reshape(m, m)

    T = np.zeros((L, L), np.float32)
    for a1 in range(L):
        for a2 in range(L):
            v = np.float32(0.0)
            if (a1 != 0) != (a2 != 0):
                v += edge_ins_del
            if a1 >= 1 and a2 >= 1:
                v += edge_costs[a1 - 1, a2 - 1]
            T[a1, a2] = v

    b2 = np.empty((m1, L, m1), np.float32)           # [k,q,l]
    for q in range(L):
        b2[:, q, :] = (A2 == q)
    TA1 = T[A1]                                       # [i,j,q]
    pmat = np.ascontiguousarray(TA1.transpose(1, 2, 0))  # [j,q,i]

    Dnm = node_costs[np.asarray(labels1)[:n][:, None], np.asarray(labels2)[:m][None, :]]
    cgrid = np.full((n1, m1), node_ins_del, np.float32)
    cgrid[:n, :m] = Dnm
    cgrid[n, m] = 0.0

    ddiag = T[A1.diagonal()[:, None], A2.diagonal()[None, :]].astype(np.float32)

    BIG = np.float32(1e4)
    cgmod = cgrid.copy()
    cgmod[:, m1 - 1] = BIG
    cgmod[n1 - 1, m1 - 1] = 0.0
    cgTmod = np.ascontiguousarray(cgrid.T)
    cgTmod[:, n1 - 1] = BIG
    cgTmod[m1 - 1, n1 - 1] = 0.0

    bf = ml_dtypes.bfloat16
    s0Tm = np.exp(-0.5 * cgTmod.astype(np.float64)).astype(bf)
    s0m = np.exp(-0.5 * cgmod.astype(np.float64)).astype(bf)
    s0 = np.exp(-0.5 * cgrid.astype(np.float64)).astype(bf)
    crit = np.stack([s0Tm, s0m], axis=1)                        # [96, 2, 96]
    g2 = np.stack([s0, ddiag.astype(bf), cgrid.astype(bf)], axis=1)

    return {
        "crit": np.ascontiguousarray(crit),
        "g2": np.ascontiguousarray(g2),
        "pm": np.ascontiguousarray(pmat.astype(bf)),
        "b2": np.ascontiguousarray(b2.astype(bf)),
    }


def run(inputs, trace=False, **spmd_kwargs):
    in_map = _host_prep(**inputs)
    if "nc" not in _NC_CACHE:
        _NC_CACHE["nc"] = _build_nc()
    nc = _NC_CACHE["nc"]
    core_ids = list(range(N_CORES))
    res = run_bass_kernel_spmd(
        nc, [dict(in_map) for _ in core_ids], core_ids, trace=trace, **spmd_kwargs
    )
    val = np.float32(res.results[0]["out"].reshape(()))
    return val, res


def kernel(**inputs) -> np.ndarray:
    val, _ = run(inputs)
    return np.asarray(val, np.float32).reshape(())


# revision 11
# speedup vs baseline: 1.3762x; 1.0003x over previous
"""Trainium2 Bass kernel for nn_GedLayer (graph edit distance forward).

The reference builds a 9216x9216 cost matrix C whose entries are a 4x4
lookup T[A1[i,j], A2[k,l]] over edge-label pairs, then computes
    ged = 0.5 * v @ (Dmat @ v) + c @ v
with v = vec(S) from a 10-iteration Sinkhorn on the 96x96 node-cost grid.

Because edge labels take only 4 values, the quadratic form factorizes into
96x96 matmuls (no 9216^2 matrix is ever formed):
    Zt[k,(q,i)] = sum_j S'[j,k] P_q[j,i]          one wide 96x96x384 matmul
    F[i,l]      = sum_qk Zt[k,(q,i)] C[k] B2_q[k,l]   4 PSUM-accum matmuls
    ged         = sum_m Cv[m]*colsum(G)[m] - 0.5*Cv[m]^2*colsum(H)[m]
with G = (0.5*F + cgrid) .* S', H = S'.^2 .* ddiag, S' = diag(R) S0, and
(R, C) from Sinkhorn run in vector form (R = 1/(S0m' C), C = 1/(S0Tm' R);
the "last scale pinned to 1" rule is implemented by baking an e_95 column
into the matvec operands so a full-tile reciprocal preserves the pin).

All device data is bf16 (PSUM accumulation stays fp32): a full-bf16
simulation of this pipeline vs the f64 oracle gives rel err ~3e-4, far
inside the 2e-2 gate. bf16 halves DMA bytes and avoids the fp32 LOW_HIGH
two-pass matmul emulation that doubles every LDWEIGHTS+MATMUL. The host
ships exp(-c/2) directly (bit-equivalent to exp-on-device at bf16) so no
activation table load or serial EXPs sit on the critical path, and the
first DMA is split across two queues because small-row DMAs here are
descriptor-rate-bound (~27ns/row), not bandwidth-bound.

Sharding: one graph pair, strictly serial Sinkhorn recursion -> the
problem is latency-bound at 96x96 scale, so the computation is replicated
on all 8 cores (SPMD) and core 0's output is returned.
"""

import numpy as np
import ml_dtypes
from contextlib import ExitStack

import concourse.bass as bass
import concourse.tile as tile
from concourse import mybir
from concourse.bass_utils import run_bass_kernel_spmd

NB_LABELS = 10
NB_EDGE_LABELS = 3
SINKHORN_ITERS = 10
L = NB_EDGE_LABELS + 1
N1 = 96
F32 = mybir.dt.float32
BF16 = mybir.dt.bfloat16
N_CORES = 8

_NC_CACHE = {}


def _legalize_waits(nc):
    """Split multi-sem waits into standalone EventSemaphore instructions
    (this walrus codegen fits one sync wait per lowered instruction)."""
    n = 0
    for f in nc.m.functions:
        for bb in f.blocks:
            out = []
            for ins in bb.instructions:
                si = ins.sync_info
                waits = list(si.on_wait) if (si and si.on_wait) else []
                if len(waits) > 1:
                    for w in waits[:-1]:
                        n += 1
                        out.append(mybir.InstEventSemaphore(
                            name=f"LW-{n}",
                            engine=ins.engine,
                            ins=[],
                            outs=[],
                            sync_info=mybir.SyncInfo(on_wait=[w], on_update=[]),
                        ))
                    si.on_wait = [waits[-1]]
                out.append(ins)
            bb.instructions = out
    return n


def _build_nc(legalize=True):
    nc = bass.Bass()
    # crit = [s0Tm | s0m] -- the Sinkhorn matvec operands, exp'd on host.
    crit_d = nc.dram_tensor("crit", [N1, 2, N1], BF16, kind="ExternalInput")
    # g2 = [s0 | ddiag | cgrid]
    g2_d = nc.dram_tensor("g2", [N1, 3, N1], BF16, kind="ExternalInput")
    pm_d = nc.dram_tensor("pm", [N1, L, N1], BF16, kind="ExternalInput")
    b2_d = nc.dram_tensor("b2", [N1, L, N1], BF16, kind="ExternalInput")
    out_d = nc.dram_tensor("out", [1, 1], F32, kind="ExternalOutput")

    mult = mybir.AluOpType.mult
    add = mybir.AluOpType.add

    with tile.TileContext(nc) as tc, ExitStack() as ctx, \
            nc.allow_low_precision("bf16 pipeline validated at 3e-4 rel err"):
        sb = ctx.enter_context(tc.tile_pool(name="sb", bufs=1))

        # crit row-split across all three DMA queues (these DMAs are
        # descriptor-rate-bound, ~30ns/row, so rows not bytes matter);
        # the bulk tensors follow behind on the same queues.
        crit = sb.tile([N1, 2, N1], BF16)
        T3 = N1 // 3
        nc.sync.dma_start(out=crit[0:T3], in_=crit_d[0:T3])
        nc.scalar.dma_start(out=crit[T3:2 * T3], in_=crit_d[T3:2 * T3])
        nc.gpsimd.dma_start(out=crit[2 * T3:N1], in_=crit_d[2 * T3:N1])
        pm = sb.tile([N1, L, N1], BF16)
        nc.sync.dma_start(out=pm[:], in_=pm_d[:])
        g2 = sb.tile([N1, 3, N1], BF16)
        nc.gpsimd.dma_start(out=g2[:], in_=g2_d[:])
        b2 = sb.tile([N1, L, N1], BF16)
        nc.sync.dma_start(out=b2[:], in_=b2_d[:])

        ones_bf = sb.tile([N1, 1], BF16)
        nc.vector.memset(ones_bf[:], 1.0)

        # Dummy activation so walrus hoists the 1.3us activation-table load
        # here (overlapping the DMA wait) instead of before the epilogue's
        # PSUM->SBUF copy.
        dmy = sb.tile([1, 1], BF16)
        nc.scalar.activation(out=dmy[:], in_=ones_bf[0:1, :],
                             func=mybir.ActivationFunctionType.Copy)

        s0Tm = crit[:, 0, :]
        s0m = crit[:, 1, :]
        s0 = g2[:, 0, :]
        dd = g2[:, 1, :]
        cg = g2[:, 2, :]

        # Sinkhorn: fresh R/C tiles per iteration (no WAR deps -> each
        # matvec and reciprocal carries exactly one semaphore wait).
        Cv = ones_bf
        Rvf = Cvf = None
        with tc.tile_pool(name="mv", bufs=4, space="PSUM") as mv:
            for it in range(SINKHORN_ITERS):
                last = it == SINKHORN_ITERS - 1
                u = mv.tile([N1, 1], F32, tag="mv")
                nc.tensor.matmul(u[:], lhsT=s0Tm, rhs=Cv[:], start=True, stop=True)
                Rv = sb.tile([N1, 1], BF16)
                nc.vector.reciprocal(out=Rv[:], in_=u[:])
                if last:
                    # f32 twin for use as tensor_scalar operands (those
                    # require f32 scalars); runs off the chain.
                    Rvf = sb.tile([N1, 1], F32)
                    nc.vector.reciprocal(out=Rvf[:], in_=u[:])
                w = mv.tile([N1, 1], F32, tag="mv")
                nc.tensor.matmul(w[:], lhsT=s0m, rhs=Rv[:], start=True, stop=True)
                if last:
                    Cvf = sb.tile([N1, 1], F32)
                    nc.vector.reciprocal(out=Cvf[:], in_=w[:])
                else:
                    Cv = sb.tile([N1, 1], BF16)
                    nc.vector.reciprocal(out=Cv[:], in_=w[:])

        # Post-Sinkhorn scalings on the vector engine (gpsimd tensor_scalar
        # is ~15x slower); G1/H products on gpsimd, off the critical path.
        sp = sb.tile([N1, N1], BF16)
        nc.vector.tensor_scalar_mul(sp[:], s0, Rvf[:])
        # b2c on the scalar engine (Copy-with-scale), parallel to vector's sp
        b2c = sb.tile([N1, L, N1], BF16)
        nc.scalar.activation(out=b2c[:].rearrange("p q l -> p (q l)"),
                             in_=b2[:].rearrange("p q l -> p (q l)"),
                             func=mybir.ActivationFunctionType.Copy, scale=Cvf[:])
        nhc2 = sb.tile([N1, 1], F32)  # -0.5 * Cv^2
        nc.vector.tensor_scalar(nhc2[:], Cvf[:], Cvf[:], -0.5, op0=mult, op1=mult)

        G1 = sb.tile([N1, N1], BF16)  # cgrid .* S'
        nc.gpsimd.tensor_mul(G1[:], cg, sp[:])
        h1 = sb.tile([N1, N1], BF16)
        nc.gpsimd.tensor_mul(h1[:], sp[:], sp[:])
        H = sb.tile([N1, N1], BF16)  # S'.^2 .* ddiag
        nc.gpsimd.tensor_mul(H[:], h1[:], dd)

        with tc.tile_pool(name="zt", bufs=1, space="PSUM") as ztp, \
                tc.tile_pool(name="fp", bufs=1, space="PSUM") as fpp, \
                tc.tile_pool(name="red", bufs=1, space="PSUM") as red:
            # Zt[k,(q,i)] = sum_j S'[j,k] P_q[j,i]
            zt_ps = ztp.tile([N1, L, N1], F32)
            nc.tensor.matmul(zt_ps[:].rearrange("p q i -> p (q i)"),
                             lhsT=sp[:], rhs=pm[:].rearrange("p q i -> p (q i)"),
                             start=True, stop=True)
            # colsum(G1) accumulates first into q_ps; G2's colsum lands on
            # top after F is ready (interleaved PSUM groups, distinct banks)
            q_ps = red.tile([N1, 1], F32, tag="q")
            nc.tensor.matmul(q_ps[:], lhsT=G1[:], rhs=ones_bf[:],
                             start=True, stop=False, skip_group_check=True)

            # PSUM -> SBUF for the F matmuls, split across two engines; two
            # tiles so each F matmul waits only on its half. The scalar half
            # is emitted first so it picks up no ordering on vector's sem.
            zt23 = sb.tile([N1, 2, N1], BF16)
            nc.scalar.activation(out=zt23[:].rearrange("p q l -> p (q l)"),
                                 in_=zt_ps[:, 2:4, :].rearrange("p q l -> p (q l)"),
                                 func=mybir.ActivationFunctionType.Copy)
            zt01 = sb.tile([N1, 2, N1], BF16)
            nc.vector.tensor_copy(out=zt01[:], in_=zt_ps[:, 0:2, :])

            f_ps = fpp.tile([N1, N1], F32)
            for q in range(L):
                zt_q = (zt01 if q < 2 else zt23)[:, q % 2, :]
                nc.tensor.matmul(f_ps[:], lhsT=zt_q, rhs=b2c[:, q, :],
                                 start=(q == 0), stop=(q == L - 1),
                                 skip_group_check=True)

            h_ps = red.tile([N1, 1], F32, tag="h")
            nc.tensor.matmul(h_ps[:], lhsT=H[:], rhs=ones_bf[:],
                             start=True, stop=True, skip_group_check=True)
            # v = colsum(H) .* (-0.5 Cv^2)  (ready before q finishes)
            v = sb.tile([N1, 1], BF16)
            nc.vector.tensor_mul(v[:], h_ps[:], nhc2[:])

            # G2 = (0.5 F) .* S' in one fused op, then its colsum
            G2 = sb.tile([N1, N1], BF16)
            nc.vector.scalar_tensor_tensor(out=G2[:], in0=f_ps[:], scalar=0.5,
                                           in1=sp[:], op0=mult, op1=mult)
            nc.tensor.matmul(q_ps[:], lhsT=G2[:], rhs=ones_bf[:],
                             start=False, stop=True, skip_group_check=True)
            wv = sb.tile([N1, 1], BF16)
            nc.vector.tensor_mul(wv[:], q_ps[:], Cvf[:])

            # ged = sum(v) + sum(wv), accumulated on the PE
            tot_ps = red.tile([1, 1], F32, tag="tot")
            nc.tensor.matmul(tot_ps[:], lhsT=v[:], rhs=ones_bf[:],
                             start=True, stop=False, skip_group_check=True)
            nc.tensor.matmul(tot_ps[:], lhsT=wv[:], rhs=ones_bf[:],
                             start=False, stop=True, skip_group_check=True)
            out_sb = sb.tile([1, 1], F32)
            nc.vector.tensor_copy(out=out_sb[:], in_=tot_ps[:])
            nc.sync.dma_start(out=out_d[:], in_=out_sb[:])

    if legalize:
        _legalize_waits(nc)
    return nc


def _host_prep(node_weights, edge_weights, A_g1, A_g2, labels1, labels2, n, m):
    n = int(n)
    m = int(m)
    n1, m1 = n + 1, m + 1
    assert n1 == N1 and m1 == N1, (n, m)

    cn = np.maximum(np.asarray(node_weights, np.float32), 0)
    ce = np.maximum(np.asarray(edge_weights, np.float32), 0)
    node_ins_del = cn[-1]
    edge_ins_del = ce[-1]
    node_costs = np.zeros((NB_LABELS, NB_LABELS), np.float32)
    node_costs[np.triu_indices(NB_LABELS, 1)] = cn[:-1]
    node_costs = node_costs + node_costs.T
    edge_costs = np.zeros((NB_EDGE_LABELS, NB_EDGE_LABELS), np.float32)
    edge_costs[np.triu_indices(NB_EDGE_LABELS, 1)] = ce[:-1]
    edge_costs = edge_costs + edge_costs.T

    A1 = np.zeros((n1, n1), np.int32)
    A1[:n, :n] = np.asarray(A_g1)[:n * n].reshape(n, n)
    A2 = np.zeros((m1, m1), np.int32)
    A2[:m, :m] = np.asarray(A_g2)[:m * m].reshape(m, m)

    T = np.zeros((L, L), np.float32)
    for a1 in range(L):
        for a2 in range(L):
            v = np.float32(0.0)
            if (a1 != 0) != (a2 != 0):
                v += edge_ins_del
            if a1 >= 1 and a2 >= 1:
                v += edge_costs[a1 - 1, a2 - 1]
            T[a1, a2] = v

    b2 = np.empty((m1, L, m1), np.float32)           # [k,q,l]
    for q in range(L):
        b2[:, q, :] = (A2 == q)
    TA1 = T[A1]                                       # [i,j,q]
    pmat = np.ascontiguousarray(TA1.transpose(1, 2, 0))  # [j,q,i]

    Dnm = node_costs[np.asarray(labels1)[:n][:, None], np.asarray(labels2)[:m][None, :]]
    cgrid = np.full((n1, m1), node_ins_del, np.float32)
    cgrid[:n, :m] = Dnm
    cgrid[n, m] = 0.0

    ddiag = T[A1.diagonal()[:, None], A2.diagonal()[None, :]].astype(np.float32)

    BIG = np.float32(1e4)
    cgmod = cgrid.copy()
    cgmod[:, m1 - 1] = BIG
    cgmod[n1 - 1, m1 - 1] = 0.0
    cgTmod = np.ascontiguousarray(cgrid.T)
    cgTmod[:, n1 - 1] = BIG
    cgTmod[m1 - 1, n1 - 1] = 0.0

    bf = ml_dtypes.bfloat16
    s0Tm = np.exp(-0.5 * cgTmod.astype(np.float64)).astype(bf)
    s0m = np.exp(-0.5 * cgmod.astype(np.float64)).astype(bf)
    s0 = np.exp(-0.5 * cgrid.astype(np.float64)).astype(bf)
    crit = np.stack([s0Tm, s0m], axis=1)                        # [96, 2, 96]
    g2 = np.stack([s0, ddiag.astype(bf), cgrid.astype(bf)], axis=1)

    return {
        "crit": np.ascontiguousarray(crit),
        "g2": np.ascontiguousarray(g2),
        "pm": np.ascontiguousarray(pmat.astype(bf)),
        "b2": np.ascontiguousarray(b2.astype(bf)),
    }


def run(inputs, trace=False, **spmd_kwargs):
    in_map = _host_prep(**inputs)
    if "nc" not in _NC_CACHE:
        _NC_CACHE["nc"] = _build_nc()
    nc = _NC_CACHE["nc"]
    core_ids = list(range(N_CORES))
    res = run_bass_kernel_spmd(
        nc, [dict(in_map) for _ in core_ids], core_ids, trace=trace, **spmd_kwargs
    )
    val = np.float32(res.results[0]["out"].reshape(()))
    return val, res


def kernel(**inputs) -> np.ndarray:
    val, _ = run(inputs)
    return np.asarray(val, np.float32).reshape(())


# revision 12
# speedup vs baseline: 1.3871x; 1.0079x over previous
"""Trainium2 Bass kernel for nn_GedLayer (graph edit distance forward).

The reference builds a 9216x9216 cost matrix C whose entries are a 4x4
lookup T[A1[i,j], A2[k,l]] over edge-label pairs, then computes
    ged = 0.5 * v @ (Dmat @ v) + c @ v
with v = vec(S) from a 10-iteration Sinkhorn on the 96x96 node-cost grid.

Because edge labels take only 4 values, the quadratic form factorizes into
96x96 matmuls (no 9216^2 matrix is ever formed):
    Zt[k,(q,i)] = sum_j S'[j,k] P_q[j,i]          one wide 96x96x384 matmul
    F[i,l]      = sum_qk Zt[k,(q,i)] C[k] B2_q[k,l]   4 PSUM-accum matmuls
    ged         = sum_m Cv[m]*colsum(G)[m] - 0.5*Cv[m]^2*colsum(H)[m]
with G = (0.5*F + cgrid) .* S', H = S'.^2 .* ddiag, S' = diag(R) S0, and
(R, C) from Sinkhorn run in vector form (R = 1/(S0m' C), C = 1/(S0Tm' R);
the "last scale pinned to 1" rule is implemented by baking an e_95 column
into the matvec operands so a full-tile reciprocal preserves the pin).

All device data is bf16 (PSUM accumulation stays fp32): a full-bf16
simulation of this pipeline vs the f64 oracle gives rel err ~3e-4, far
inside the 2e-2 gate. bf16 halves DMA bytes and avoids the fp32 LOW_HIGH
two-pass matmul emulation that doubles every LDWEIGHTS+MATMUL. The host
ships exp(-c/2) directly (bit-equivalent to exp-on-device at bf16) so no
activation table load or serial EXPs sit on the critical path, and the
first DMA is split across two queues because small-row DMAs here are
descriptor-rate-bound (~27ns/row), not bandwidth-bound.

Sharding: one graph pair, strictly serial Sinkhorn recursion -> the
problem is latency-bound at 96x96 scale, so the computation is replicated
on all 8 cores (SPMD) and core 0's output is returned.
"""

import numpy as np
import ml_dtypes
from contextlib import ExitStack

import concourse.bass as bass
import concourse.tile as tile
from concourse import mybir
from concourse.bass_utils import run_bass_kernel_spmd

NB_LABELS = 10
NB_EDGE_LABELS = 3
SINKHORN_ITERS = 10
L = NB_EDGE_LABELS + 1
N1 = 96
F32 = mybir.dt.float32
BF16 = mybir.dt.bfloat16
N_CORES = 8

_NC_CACHE = {}


def _legalize_waits(nc):
    """Split multi-sem waits into standalone EventSemaphore instructions
    (this walrus codegen fits one sync wait per lowered instruction)."""
    n = 0
    for f in nc.m.functions:
        for bb in f.blocks:
            out = []
            for ins in bb.instructions:
                si = ins.sync_info
                waits = list(si.on_wait) if (si and si.on_wait) else []
                if len(waits) > 1:
                    for w in waits[:-1]:
                        n += 1
                        out.append(mybir.InstEventSemaphore(
                            name=f"LW-{n}",
                            engine=ins.engine,
                            ins=[],
                            outs=[],
                            sync_info=mybir.SyncInfo(on_wait=[w], on_update=[]),
                        ))
                    si.on_wait = [waits[-1]]
                out.append(ins)
            bb.instructions = out
    return n


def _build_nc(legalize=True):
    nc = bass.Bass()
    # crit = [s0Tm | s0m] -- the Sinkhorn matvec operands, exp'd on host.
    crit_d = nc.dram_tensor("crit", [N1, 2, N1], BF16, kind="ExternalInput")
    # g2 = [s0 | ddiag | cgrid]
    g2_d = nc.dram_tensor("g2", [N1, 3, N1], BF16, kind="ExternalInput")
    pm_d = nc.dram_tensor("pm", [N1, L, N1], BF16, kind="ExternalInput")
    b2_d = nc.dram_tensor("b2", [N1, L, N1], BF16, kind="ExternalInput")
    out_d = nc.dram_tensor("out", [1, 1], F32, kind="ExternalOutput")

    mult = mybir.AluOpType.mult
    add = mybir.AluOpType.add

    with tile.TileContext(nc) as tc, ExitStack() as ctx, \
            nc.allow_low_precision("bf16 pipeline validated at 3e-4 rel err"):
        sb = ctx.enter_context(tc.tile_pool(name="sb", bufs=1))

        # crit row-split across all three DMA queues (these DMAs are
        # descriptor-rate-bound, ~30ns/row, so rows not bytes matter);
        # the bulk tensors follow behind on the same queues.
        crit = sb.tile([N1, 2, N1], BF16)
        T3 = N1 // 3
        nc.sync.dma_start(out=crit[0:T3], in_=crit_d[0:T3])
        nc.scalar.dma_start(out=crit[T3:2 * T3], in_=crit_d[T3:2 * T3])
        nc.gpsimd.dma_start(out=crit[2 * T3:N1], in_=crit_d[2 * T3:N1])
        pm = sb.tile([N1, L, N1], BF16)
        nc.sync.dma_start(out=pm[:], in_=pm_d[:])
        g2 = sb.tile([N1, 3, N1], BF16)
        nc.gpsimd.dma_start(out=g2[:], in_=g2_d[:])
        b2 = sb.tile([N1, L, N1], BF16)
        nc.sync.dma_start(out=b2[:], in_=b2_d[:])

        ones_bf = sb.tile([N1, 1], BF16)
        nc.vector.memset(ones_bf[:], 1.0)

        # Dummy activation so walrus hoists the 1.3us activation-table load
        # here (overlapping the DMA wait) instead of before the epilogue's
        # PSUM->SBUF copy.
        dmy = sb.tile([1, 1], BF16)
        nc.scalar.activation(out=dmy[:], in_=ones_bf[0:1, :],
                             func=mybir.ActivationFunctionType.Copy)

        s0Tm = crit[:, 0, :]
        s0m = crit[:, 1, :]
        s0 = g2[:, 0, :]
        dd = g2[:, 1, :]
        cg = g2[:, 2, :]

        # Sinkhorn: fresh R/C tiles per iteration (no WAR deps -> each
        # matvec and reciprocal carries exactly one semaphore wait).
        Cv = ones_bf
        Rvf = Cvf = None
        with tc.tile_pool(name="mv", bufs=4, space="PSUM") as mv:
            for it in range(SINKHORN_ITERS):
                last = it == SINKHORN_ITERS - 1
                u = mv.tile([N1, 1], F32, tag="mv")
                nc.tensor.matmul(u[:], lhsT=s0Tm, rhs=Cv[:], start=True, stop=True)
                Rv = sb.tile([N1, 1], BF16)
                nc.vector.reciprocal(out=Rv[:], in_=u[:])
                if last:
                    # f32 twin for use as tensor_scalar operands (those
                    # require f32 scalars); runs off the chain.
                    Rvf = sb.tile([N1, 1], F32)
                    nc.vector.reciprocal(out=Rvf[:], in_=u[:])
                w = mv.tile([N1, 1], F32, tag="mv")
                nc.tensor.matmul(w[:], lhsT=s0m, rhs=Rv[:], start=True, stop=True)
                if last:
                    Cvf = sb.tile([N1, 1], F32)
                    nc.vector.reciprocal(out=Cvf[:], in_=w[:])
                else:
                    Cv = sb.tile([N1, 1], BF16)
                    nc.vector.reciprocal(out=Cv[:], in_=w[:])

        # Post-Sinkhorn scalings on the vector engine (gpsimd tensor_scalar
        # is ~15x slower); G1/H products on gpsimd, off the critical path.
        sp = sb.tile([N1, N1], BF16)
        nc.vector.tensor_scalar_mul(sp[:], s0, Rvf[:])
        # b2c on the scalar engine (Copy-with-scale), parallel to vector's sp
        b2c = sb.tile([N1, L, N1], BF16)
        nc.scalar.activation(out=b2c[:].rearrange("p q l -> p (q l)"),
                             in_=b2[:].rearrange("p q l -> p (q l)"),
                             func=mybir.ActivationFunctionType.Copy, scale=Cvf[:])
        nhc2 = sb.tile([N1, 1], F32)  # -0.5 * Cv^2
        nc.vector.tensor_scalar(nhc2[:], Cvf[:], Cvf[:], -0.5, op0=mult, op1=mult)

        G1 = sb.tile([N1, N1], BF16)  # cgrid .* S'
        nc.gpsimd.tensor_mul(G1[:], cg, sp[:])
        h1 = sb.tile([N1, N1], BF16)
        nc.gpsimd.tensor_mul(h1[:], sp[:], sp[:])
        H = sb.tile([N1, N1], BF16)  # S'.^2 .* ddiag
        nc.gpsimd.tensor_mul(H[:], h1[:], dd)

        with tc.tile_pool(name="zt", bufs=1, space="PSUM") as ztp, \
                tc.tile_pool(name="fp", bufs=1, space="PSUM") as fpp, \
                tc.tile_pool(name="red", bufs=1, space="PSUM") as red:
            # Zt[k,(q,i)] = sum_j S'[j,k] P_q[j,i], split into two PSUM
            # tiles so the two PSUM->SBUF copy engines don't serialize
            # (Tile chains readers of a single PSUM tile).
            zt_psA = ztp.tile([N1, 2, N1], F32, tag="a")
            nc.tensor.matmul(zt_psA[:].rearrange("p q i -> p (q i)"),
                             lhsT=sp[:],
                             rhs=pm[:, 0:2, :].rearrange("p q i -> p (q i)"),
                             start=True, stop=True)
            zt_psB = ztp.tile([N1, 2, N1], F32, tag="b")
            nc.tensor.matmul(zt_psB[:].rearrange("p q i -> p (q i)"),
                             lhsT=sp[:],
                             rhs=pm[:, 2:4, :].rearrange("p q i -> p (q i)"),
                             start=True, stop=True)

            zt01 = sb.tile([N1, 2, N1], BF16)
            nc.vector.tensor_copy(out=zt01[:], in_=zt_psA[:])
            zt23 = sb.tile([N1, 2, N1], BF16)
            nc.scalar.activation(out=zt23[:].rearrange("p q l -> p (q l)"),
                                 in_=zt_psB[:].rearrange("p q l -> p (q l)"),
                                 func=mybir.ActivationFunctionType.Copy)

            f_ps = fpp.tile([N1, N1], F32)
            for q in range(L):
                zt_q = (zt01 if q < 2 else zt23)[:, q % 2, :]
                nc.tensor.matmul(f_ps[:], lhsT=zt_q, rhs=b2c[:, q, :],
                                 start=(q == 0), stop=(q == L - 1),
                                 skip_group_check=True)

            # colsums after F so they don't delay it on the PE queue;
            # G1's lands in q_ps first, G2's accumulates on top.
            q_ps = red.tile([N1, 1], F32, tag="q")
            nc.tensor.matmul(q_ps[:], lhsT=G1[:], rhs=ones_bf[:],
                             start=True, stop=False, skip_group_check=True)
            h_ps = red.tile([N1, 1], F32, tag="h")
            nc.tensor.matmul(h_ps[:], lhsT=H[:], rhs=ones_bf[:],
                             start=True, stop=True, skip_group_check=True)
            # v = colsum(H) .* (-0.5 Cv^2)  (ready before q finishes)
            v = sb.tile([N1, 1], BF16)
            nc.vector.tensor_mul(v[:], h_ps[:], nhc2[:])

            # G2 = (0.5 F) .* S' in one fused op, then its colsum
            G2 = sb.tile([N1, N1], BF16)
            nc.vector.scalar_tensor_tensor(out=G2[:], in0=f_ps[:], scalar=0.5,
                                           in1=sp[:], op0=mult, op1=mult)
            nc.tensor.matmul(q_ps[:], lhsT=G2[:], rhs=ones_bf[:],
                             start=False, stop=True, skip_group_check=True)
            wv = sb.tile([N1, 1], BF16)
            nc.vector.tensor_mul(wv[:], q_ps[:], Cvf[:])

            # ged = sum(v) + sum(wv), accumulated on the PE
            tot_ps = red.tile([1, 1], F32, tag="tot")
            nc.tensor.matmul(tot_ps[:], lhsT=v[:], rhs=ones_bf[:],
                             start=True, stop=False, skip_group_check=True)
            nc.tensor.matmul(tot_ps[:], lhsT=wv[:], rhs=ones_bf[:],
                             start=False, stop=True, skip_group_check=True)
            out_sb = sb.tile([1, 1], F32)
            nc.vector.tensor_copy(out=out_sb[:], in_=tot_ps[:])
            nc.sync.dma_start(out=out_d[:], in_=out_sb[:])

    if legalize:
        _legalize_waits(nc)
    return nc


def _host_prep(node_weights, edge_weights, A_g1, A_g2, labels1, labels2, n, m):
    n = int(n)
    m = int(m)
    n1, m1 = n + 1, m + 1
    assert n1 == N1 and m1 == N1, (n, m)

    cn = np.maximum(np.asarray(node_weights, np.float32), 0)
    ce = np.maximum(np.asarray(edge_weights, np.float32), 0)
    node_ins_del = cn[-1]
    edge_ins_del = ce[-1]
    node_costs = np.zeros((NB_LABELS, NB_LABELS), np.float32)
    node_costs[np.triu_indices(NB_LABELS, 1)] = cn[:-1]
    node_costs = node_costs + node_costs.T
    edge_costs = np.zeros((NB_EDGE_LABELS, NB_EDGE_LABELS), np.float32)
    edge_costs[np.triu_indices(NB_EDGE_LABELS, 1)] = ce[:-1]
    edge_costs = edge_costs + edge_costs.T

    A1 = np.zeros((n1, n1), np.int32)
    A1[:n, :n] = np.asarray(A_g1)[:n * n].reshape(n, n)
    A2 = np.zeros((m1, m1), np.int32)
    A2[:m, :m] = np.asarray(A_g2)[:m * m].reshape(m, m)

    T = np.zeros((L, L), np.float32)
    for a1 in range(L):
        for a2 in range(L):
            v = np.float32(0.0)
            if (a1 != 0) != (a2 != 0):
                v += edge_ins_del
            if a1 >= 1 and a2 >= 1:
                v += edge_costs[a1 - 1, a2 - 1]
            T[a1, a2] = v

    b2 = np.empty((m1, L, m1), np.float32)           # [k,q,l]
    for q in range(L):
        b2[:, q, :] = (A2 == q)
    TA1 = T[A1]                                       # [i,j,q]
    pmat = np.ascontiguousarray(TA1.transpose(1, 2, 0))  # [j,q,i]

    Dnm = node_costs[np.asarray(labels1)[:n][:, None], np.asarray(labels2)[:m][None, :]]
    cgrid = np.full((n1, m1), node_ins_del, np.float32)
    cgrid[:n, :m] = Dnm
    cgrid[n, m] = 0.0

    ddiag = T[A1.diagonal()[:, None], A2.diagonal()[None, :]].astype(np.float32)

    BIG = np.float32(1e4)
    cgmod = cgrid.copy()
    cgmod[:, m1 - 1] = BIG
    cgmod[n1 - 1, m1 - 1] = 0.0
    cgTmod = np.ascontiguousarray(cgrid.T)
    cgTmod[:, n1 - 1] = BIG
    cgTmod[m1 - 1, n1 - 1] = 0.0

    bf = ml_dtypes.bfloat16
    s0Tm = np.exp(-0.5 * cgTmod.astype(np.float64)).astype(bf)
    s0m = np.exp(-0.5 * cgmod.astype(np.float64)).astype(bf)
    s0 = np.exp(-0.5 * cgrid.astype(np.float64)).astype(bf)
    crit = np.stack([s0Tm, s0m], axis=1)                        # [96, 2, 96]
    g2 = np.stack([s0, ddiag.astype(bf), cgrid.astype(bf)], axis=1)

    return {
        "crit": np.ascontiguousarray(crit),
        "g2": np.ascontiguousarray(g2),
        "pm": np.ascontiguousarray(pmat.astype(bf)),
        "b2": np.ascontiguousarray(b2.astype(bf)),
    }


def run(inputs, trace=False, **spmd_kwargs):
    in_map = _host_prep(**inputs)
    if "nc" not in _NC_CACHE:
        _NC_CACHE["nc"] = _build_nc()
    nc = _NC_CACHE["nc"]
    core_ids = list(range(N_CORES))
    res = run_bass_kernel_spmd(
        nc, [dict(in_map) for _ in core_ids], core_ids, trace=trace, **spmd_kwargs
    )
    val = np.float32(res.results[0]["out"].reshape(()))
    return val, res


def kernel(**inputs) -> np.ndarray:
    val, _ = run(inputs)
    return np.asarray(val, np.float32).reshape(())


# revision 13
# speedup vs baseline: 1.3911x; 1.0029x over previous
"""Trainium2 Bass kernel for nn_GedLayer (graph edit distance forward).

The reference builds a 9216x9216 cost matrix C whose entries are a 4x4
lookup T[A1[i,j], A2[k,l]] over edge-label pairs, then computes
    ged = 0.5 * v @ (Dmat @ v) + c @ v
with v = vec(S) from a 10-iteration Sinkhorn on the 96x96 node-cost grid.

Because edge labels take only 4 values, the quadratic form factorizes into
96x96 matmuls (no 9216^2 matrix is ever formed):
    Zt[k,(q,i)] = sum_j S'[j,k] P_q[j,i]          one wide 96x96x384 matmul
    F[i,l]      = sum_qk Zt[k,(q,i)] C[k] B2_q[k,l]   4 PSUM-accum matmuls
    ged         = sum_m Cv[m]*colsum(G)[m] - 0.5*Cv[m]^2*colsum(H)[m]
with G = (0.5*F + cgrid) .* S', H = S'.^2 .* ddiag, S' = diag(R) S0, and
(R, C) from Sinkhorn run in vector form (R = 1/(S0m' C), C = 1/(S0Tm' R);
the "last scale pinned to 1" rule is implemented by baking an e_95 column
into the matvec operands so a full-tile reciprocal preserves the pin).

All device data is bf16 (PSUM accumulation stays fp32): a full-bf16
simulation of this pipeline vs the f64 oracle gives rel err ~3e-4, far
inside the 2e-2 gate. bf16 halves DMA bytes and avoids the fp32 LOW_HIGH
two-pass matmul emulation that doubles every LDWEIGHTS+MATMUL. The host
ships exp(-c/2) directly (bit-equivalent to exp-on-device at bf16) so no
activation table load or serial EXPs sit on the critical path, and the
first DMA is split across two queues because small-row DMAs here are
descriptor-rate-bound (~27ns/row), not bandwidth-bound.

Sharding: one graph pair, strictly serial Sinkhorn recursion -> the
problem is latency-bound at 96x96 scale, so the computation is replicated
on all 8 cores (SPMD) and core 0's output is returned.
"""

import numpy as np
import ml_dtypes
from contextlib import ExitStack

import concourse.bass as bass
import concourse.tile as tile
from concourse import mybir
from concourse.bass_utils import run_bass_kernel_spmd

NB_LABELS = 10
NB_EDGE_LABELS = 3
SINKHORN_ITERS = 10
L = NB_EDGE_LABELS + 1
N1 = 96
F32 = mybir.dt.float32
BF16 = mybir.dt.bfloat16
N_CORES = 8

_NC_CACHE = {}


def _legalize_waits(nc):
    """Split multi-sem waits into standalone EventSemaphore instructions
    (this walrus codegen fits one sync wait per lowered instruction)."""
    n = 0
    for f in nc.m.functions:
        for bb in f.blocks:
            out = []
            for ins in bb.instructions:
                si = ins.sync_info
                waits = list(si.on_wait) if (si and si.on_wait) else []
                if len(waits) > 1:
                    for w in waits[:-1]:
                        n += 1
                        out.append(mybir.InstEventSemaphore(
                            name=f"LW-{n}",
                            engine=ins.engine,
                            ins=[],
                            outs=[],
                            sync_info=mybir.SyncInfo(on_wait=[w], on_update=[]),
                        ))
                    si.on_wait = [waits[-1]]
                out.append(ins)
            bb.instructions = out
    return n


def _build_nc(legalize=True):
    nc = bass.Bass()
    # crit = [s0Tm | s0m] -- the Sinkhorn matvec operands, exp'd on host.
    crit_d = nc.dram_tensor("crit", [N1, 2, N1], BF16, kind="ExternalInput")
    # g2 = [s0 | ddiag | cgrid]
    g2_d = nc.dram_tensor("g2", [N1, 3, N1], BF16, kind="ExternalInput")
    pm_d = nc.dram_tensor("pm", [N1, L, N1], BF16, kind="ExternalInput")
    b2_d = nc.dram_tensor("b2", [N1, L, N1], BF16, kind="ExternalInput")
    out_d = nc.dram_tensor("out", [1, 1], F32, kind="ExternalOutput")

    mult = mybir.AluOpType.mult
    add = mybir.AluOpType.add

    with tile.TileContext(nc) as tc, ExitStack() as ctx, \
            nc.allow_low_precision("bf16 pipeline validated at 3e-4 rel err"):
        sb = ctx.enter_context(tc.tile_pool(name="sb", bufs=1))

        # crit row-split across all three DMA queues (these DMAs are
        # descriptor-rate-bound, ~30ns/row, so rows not bytes matter);
        # the bulk tensors follow behind on the same queues.
        crit = sb.tile([N1, 2, N1], BF16)
        T3 = N1 // 3
        nc.sync.dma_start(out=crit[0:T3], in_=crit_d[0:T3])
        nc.scalar.dma_start(out=crit[T3:2 * T3], in_=crit_d[T3:2 * T3])
        nc.gpsimd.dma_start(out=crit[2 * T3:N1], in_=crit_d[2 * T3:N1])
        pm = sb.tile([N1, L, N1], BF16)
        nc.sync.dma_start(out=pm[:], in_=pm_d[:])
        g2 = sb.tile([N1, 3, N1], BF16)
        nc.gpsimd.dma_start(out=g2[:], in_=g2_d[:])
        b2 = sb.tile([N1, L, N1], BF16)
        nc.sync.dma_start(out=b2[:], in_=b2_d[:])

        ones_bf = sb.tile([N1, 1], BF16)
        nc.vector.memset(ones_bf[:], 1.0)
        neg_ones = sb.tile([N1, 1], BF16)
        nc.vector.memset(neg_ones[:], -1.0)

        # Dummy activation so walrus hoists the 1.3us activation-table load
        # here (overlapping the DMA wait) instead of before the epilogue's
        # PSUM->SBUF copy.
        dmy = sb.tile([1, 1], BF16)
        nc.scalar.activation(out=dmy[:], in_=ones_bf[0:1, :],
                             func=mybir.ActivationFunctionType.Copy)

        s0Tm = crit[:, 0, :]
        s0m = crit[:, 1, :]
        s0 = g2[:, 0, :]
        dd = g2[:, 1, :]
        cg = g2[:, 2, :]

        # Sinkhorn: fresh R/C tiles per iteration (no WAR deps -> each
        # matvec and reciprocal carries exactly one semaphore wait).
        Cv = ones_bf
        Rvf = Cvf = None
        with tc.tile_pool(name="mv", bufs=4, space="PSUM") as mv:
            for it in range(SINKHORN_ITERS):
                last = it == SINKHORN_ITERS - 1
                u = mv.tile([N1, 1], F32, tag="mv")
                nc.tensor.matmul(u[:], lhsT=s0Tm, rhs=Cv[:], start=True, stop=True)
                Rv = sb.tile([N1, 1], BF16)
                nc.vector.reciprocal(out=Rv[:], in_=u[:])
                if last:
                    # f32 twin for use as tensor_scalar operands (those
                    # require f32 scalars); runs off the chain.
                    Rvf = sb.tile([N1, 1], F32)
                    nc.vector.reciprocal(out=Rvf[:], in_=u[:])
                w = mv.tile([N1, 1], F32, tag="mv")
                nc.tensor.matmul(w[:], lhsT=s0m, rhs=Rv[:], start=True, stop=True)
                if last:
                    Cvf = sb.tile([N1, 1], F32)
                    nc.vector.reciprocal(out=Cvf[:], in_=w[:])
                else:
                    Cv = sb.tile([N1, 1], BF16)
                    nc.vector.reciprocal(out=Cv[:], in_=w[:])

        # Post-Sinkhorn scalings on the vector engine (gpsimd tensor_scalar
        # is ~15x slower); G1/H products on gpsimd, off the critical path.
        sp = sb.tile([N1, N1], BF16)
        nc.vector.tensor_scalar_mul(sp[:], s0, Rvf[:])
        # H path early on vector so its colsum matmul never blocks F
        h1 = sb.tile([N1, N1], BF16)
        nc.vector.tensor_mul(h1[:], sp[:], sp[:])
        H = sb.tile([N1, N1], BF16)  # S'.^2 .* ddiag
        nc.vector.tensor_mul(H[:], h1[:], dd)
        # 0.5*Cv^2 via Square on scalar; the minus sign rides neg_ones below
        nhc2p = sb.tile([N1, 1], F32)
        nc.scalar.activation(out=nhc2p[:], in_=Cvf[:],
                             func=mybir.ActivationFunctionType.Square,
                             scale=float(np.sqrt(0.5)))
        G1 = sb.tile([N1, N1], BF16)  # cgrid .* S'
        nc.gpsimd.tensor_mul(G1[:], cg, sp[:])

        with tc.tile_pool(name="zt", bufs=1, space="PSUM") as ztp, \
                tc.tile_pool(name="fp", bufs=1, space="PSUM") as fpp, \
                tc.tile_pool(name="red", bufs=1, space="PSUM") as red:
            # Zt[k,(q,i)] = sum_j S'[j,k] P_q[j,i], split into two PSUM
            # tiles so the two PSUM->SBUF copy engines don't serialize
            # (Tile chains readers of a single PSUM tile).
            zt_psA = ztp.tile([N1, 2, N1], F32, tag="a")
            nc.tensor.matmul(zt_psA[:].rearrange("p q i -> p (q i)"),
                             lhsT=sp[:],
                             rhs=pm[:, 0:2, :].rearrange("p q i -> p (q i)"),
                             start=True, stop=True)
            zt_psB = ztp.tile([N1, 2, N1], F32, tag="b")
            nc.tensor.matmul(zt_psB[:].rearrange("p q i -> p (q i)"),
                             lhsT=sp[:],
                             rhs=pm[:, 2:4, :].rearrange("p q i -> p (q i)"),
                             start=True, stop=True)

            # PSUM->SBUF copies also fold in the diag(Cv) scaling, so F
            # can consume the raw b2 indicator tables directly.
            zt01 = sb.tile([N1, 2, N1], BF16)
            nc.vector.tensor_scalar_mul(zt01[:].rearrange("p q l -> p (q l)"),
                                        zt_psA[:].rearrange("p q l -> p (q l)"),
                                        Cvf[:])
            zt23 = sb.tile([N1, 2, N1], BF16)
            nc.scalar.activation(out=zt23[:].rearrange("p q l -> p (q l)"),
                                 in_=zt_psB[:].rearrange("p q l -> p (q l)"),
                                 func=mybir.ActivationFunctionType.Copy,
                                 scale=Cvf[:])

            f_ps = fpp.tile([N1, N1], F32)
            for q in range(L):
                zt_q = (zt01 if q < 2 else zt23)[:, q % 2, :]
                nc.tensor.matmul(f_ps[:], lhsT=zt_q, rhs=b2[:, q, :],
                                 start=(q == 0), stop=(q == L - 1),
                                 skip_group_check=True)

            # colsums after F so they don't delay it on the PE queue;
            # G1's lands in q_ps first, G2's accumulates on top.
            q_ps = red.tile([N1, 1], F32, tag="q")
            nc.tensor.matmul(q_ps[:], lhsT=G1[:], rhs=ones_bf[:],
                             start=True, stop=False, skip_group_check=True)
            h_ps = red.tile([N1, 1], F32, tag="h")
            nc.tensor.matmul(h_ps[:], lhsT=H[:], rhs=ones_bf[:],
                             start=True, stop=True, skip_group_check=True)
            # v = colsum(H) .* (0.5 Cv^2)  (ready before q finishes)
            v = sb.tile([N1, 1], BF16)
            nc.scalar.activation(out=v[:], in_=h_ps[:],
                                 func=mybir.ActivationFunctionType.Copy,
                                 scale=nhc2p[:])

            # G2 = (0.5 F) .* S' in one fused op, then its colsum
            G2 = sb.tile([N1, N1], BF16)
            nc.vector.scalar_tensor_tensor(out=G2[:], in0=f_ps[:], scalar=0.5,
                                           in1=sp[:], op0=mult, op1=mult)
            nc.tensor.matmul(q_ps[:], lhsT=G2[:], rhs=ones_bf[:],
                             start=False, stop=True, skip_group_check=True)
            wv = sb.tile([N1, 1], BF16)
            nc.scalar.activation(out=wv[:], in_=q_ps[:],
                                 func=mybir.ActivationFunctionType.Copy,
                                 scale=Cvf[:])

            # ged = sum(wv) - sum(v), accumulated on the PE
            tot_ps = red.tile([1, 1], F32, tag="tot")
            nc.tensor.matmul(tot_ps[:], lhsT=v[:], rhs=neg_ones[:],
                             start=True, stop=False, skip_group_check=True)
            nc.tensor.matmul(tot_ps[:], lhsT=wv[:], rhs=ones_bf[:],
                             start=False, stop=True, skip_group_check=True)
            out_sb = sb.tile([1, 1], F32)
            nc.scalar.activation(out=out_sb[:], in_=tot_ps[:],
                                 func=mybir.ActivationFunctionType.Copy)
            nc.sync.dma_start(out=out_d[:], in_=out_sb[:])

    if legalize:
        _legalize_waits(nc)
    return nc


def _host_prep(node_weights, edge_weights, A_g1, A_g2, labels1, labels2, n, m):
    n = int(n)
    m = int(m)
    n1, m1 = n + 1, m + 1
    assert n1 == N1 and m1 == N1, (n, m)

    cn = np.maximum(np.asarray(node_weights, np.float32), 0)
    ce = np.maximum(np.asarray(edge_weights, np.float32), 0)
    node_ins_del = cn[-1]
    edge_ins_del = ce[-1]
    node_costs = np.zeros((NB_LABELS, NB_LABELS), np.float32)
    node_costs[np.triu_indices(NB_LABELS, 1)] = cn[:-1]
    node_costs = node_costs + node_costs.T
    edge_costs = np.zeros((NB_EDGE_LABELS, NB_EDGE_LABELS), np.float32)
    edge_costs[np.triu_indices(NB_EDGE_LABELS, 1)] = ce[:-1]
    edge_costs = edge_costs + edge_costs.T

    A1 = np.zeros((n1, n1), np.int32)
    A1[:n, :n] = np.asarray(A_g1)[:n * n].reshape(n, n)
    A2 = np.zeros((m1, m1), np.int32)
    A2[:m, :m] = np.asarray(A_g2)[:m * m].reshape(m, m)

    T = np.zeros((L, L), np.float32)
    for a1 in range(L):
        for a2 in range(L):
            v = np.float32(0.0)
            if (a1 != 0) != (a2 != 0):
                v += edge_ins_del
            if a1 >= 1 and a2 >= 1:
                v += edge_costs[a1 - 1, a2 - 1]
            T[a1, a2] = v

    b2 = np.empty((m1, L, m1), np.float32)           # [k,q,l]
    for q in range(L):
        b2[:, q, :] = (A2 == q)
    TA1 = T[A1]                                       # [i,j,q]
    pmat = np.ascontiguousarray(TA1.transpose(1, 2, 0))  # [j,q,i]

    Dnm = node_costs[np.asarray(labels1)[:n][:, None], np.asarray(labels2)[:m][None, :]]
    cgrid = np.full((n1, m1), node_ins_del, np.float32)
    cgrid[:n, :m] = Dnm
    cgrid[n, m] = 0.0

    ddiag = T[A1.diagonal()[:, None], A2.diagonal()[None, :]].astype(np.float32)

    BIG = np.float32(1e4)
    cgmod = cgrid.copy()
    cgmod[:, m1 - 1] = BIG
    cgmod[n1 - 1, m1 - 1] = 0.0
    cgTmod = np.ascontiguousarray(cgrid.T)
    cgTmod[:, n1 - 1] = BIG
    cgTmod[m1 - 1, n1 - 1] = 0.0

    bf = ml_dtypes.bfloat16
    s0Tm = np.exp(-0.5 * cgTmod.astype(np.float64)).astype(bf)
    s0m = np.exp(-0.5 * cgmod.astype(np.float64)).astype(bf)
    s0 = np.exp(-0.5 * cgrid.astype(np.float64)).astype(bf)
    crit = np.stack([s0Tm, s0m], axis=1)                        # [96, 2, 96]
    g2 = np.stack([s0, ddiag.astype(bf), cgrid.astype(bf)], axis=1)

    return {
        "crit": np.ascontiguousarray(crit),
        "g2": np.ascontiguousarray(g2),
        "pm": np.ascontiguousarray(pmat.astype(bf)),
        "b2": np.ascontiguousarray(b2.astype(bf)),
    }


def run(inputs, trace=False, **spmd_kwargs):
    in_map = _host_prep(**inputs)
    if "nc" not in _NC_CACHE:
        _NC_CACHE["nc"] = _build_nc()
    nc = _NC_CACHE["nc"]
    core_ids = list(range(N_CORES))
    res = run_bass_kernel_spmd(
        nc, [dict(in_map) for _ in core_ids], core_ids, trace=trace, **spmd_kwargs
    )
    val = np.float32(res.results[0]["out"].reshape(()))
    return val, res


def kernel(**inputs) -> np.ndarray:
    val, _ = run(inputs)
    return np.asarray(val, np.float32).reshape(())
